# revision 1
# baseline (speedup 1.0000x reference)
"""GCN block (GCNConv + LayerNorm + ReLU) on 8 Trainium2 NeuronCores.

Strategy (matches the "shard nodes / partition edges by destination" hint):
  - out = LN(A_norm @ (x @ W^T) + b) = LN((A_norm @ x) @ W^T + b): aggregate
    raw features first (A_norm commutes with the linear map), so the random
    gather runs on node-major x and no transposes are needed anywhere.
  - Destination nodes are sharded contiguously across the 8 cores
    (6250 rows each); each core processes the edges that point into its
    shard.  x is replicated in every core's DRAM as two bf16 gather tables
    (even/odd node rows, so row indices fit dma_gather's int16 indices).
  - Edges are bucketed per 128-destination-node block and padded to whole
    128-edge tiles; multi-block chunks of source rows are fetched with one
    dma_gather per table (output lands tile-major: row j -> partition j%128,
    chunk j//128).  For each 128-edge tile a [128e x 128d] selection matrix
    S (S[e, d] = norm_e if dst_e == d) is built with one fused DVE
    tensor_scalar (iota == dstcol) * norm; the scatter-add is then
    G_cblk^T @ S accumulated in PSUM over the block's tiles, which directly
    yields agg^T laid out as [channel, dst] — exactly the stationary operand
    the W-matmul wants.  agg^T @ W^T gives [dst, out_ch] node-major, and
    bias + LayerNorm + ReLU are fused on DVE/ACT before a contiguous store.
"""

import math
import sys

sys.path.insert(0, "/opt/trn_rl_repo")

import numpy as np
import ml_dtypes

N_NODES = 50000
WIDTH = 256
N_CORES = 8
NODES_PER_CORE = N_NODES // N_CORES  # 6250
P = 128
N_BLOCKS = math.ceil(NODES_PER_CORE / P)  # 49 (last block has 106 rows)
LN_EPS = 1e-5
HALF = N_NODES // 2  # rows per gather table

USE_BF16 = True
GATHER_TILE_CAP = 8  # max tiles (128 idxs each) per dma_gather call (HW limit 1024)


def _preprocess(edge_index):
    """Bucket messages by (core, dst-block, src-parity table), pad each bucket
    to whole 128-edge tiles.

    Processing tile order: per block, even-table tiles then odd-table tiles.
    Gather order: even tiles of all blocks concatenated (ditto odd).
    Returns (TL, TH, dstcol[8,P,Ttot], normv[8,P,Ttot],
             idxe[8,128,8*sum(TL)] i16, idxo[8,128,8*sum(TH)] i16).
    """
    src = np.asarray(edge_index[0]).astype(np.int64)
    dst = np.asarray(edge_index[1]).astype(np.int64)
    loops = np.arange(N_NODES, dtype=np.int64)
    msrc = np.concatenate([src, loops])
    mdst = np.concatenate([dst, loops])

    deg = np.bincount(mdst, minlength=N_NODES).astype(np.float64)
    dinv = 1.0 / np.sqrt(deg)  # deg >= 1 thanks to self loops
    norm = (dinv[msrc] * dinv[mdst]).astype(np.float32)

    core = mdst // NODES_PER_CORE
    r = mdst % NODES_PER_CORE
    blk = np.minimum(r // P, N_BLOCKS - 1)
    dcol = (r - blk * P).astype(np.float32)
    tab = msrc & 1
    gbin = (core * N_BLOCKS + blk) * 2 + tab

    order = np.argsort(gbin, kind="stable")
    msrc, norm, dcol, gbin = msrc[order], norm[order], dcol[order], gbin[order]

    cnt = np.bincount(gbin, minlength=N_CORES * N_BLOCKS * 2).reshape(
        N_CORES, N_BLOCKS, 2
    )
    TL = [int(math.ceil(int(cnt[:, b, 0].max()) / P)) for b in range(N_BLOCKS)]
    TH = [int(math.ceil(int(cnt[:, b, 1].max()) / P)) for b in range(N_BLOCKS)]
    sTL, sTH = sum(TL), sum(TH)
    Ttot = sTL + sTH
    # tile offsets
    EOFF = np.concatenate([[0], np.cumsum(TL)])  # even gather order
    OOFF = np.concatenate([[0], np.cumsum(TH)])  # odd gather order
    TOFF = np.concatenate([[0], np.cumsum(np.asarray(TL) + np.asarray(TH))])

    dstcol = np.zeros((N_CORES, P, Ttot), np.float32)
    normv = np.zeros((N_CORES, P, Ttot), np.float32)
    idxe_flat = np.zeros((N_CORES, sTL * P), np.int16)
    idxo_flat = np.zeros((N_CORES, sTH * P), np.int16)

    starts = np.concatenate([[0], np.cumsum(cnt.ravel())])[:-1]
    j = np.arange(len(gbin)) - starts[gbin]  # index within bucket
    c = gbin // (N_BLOCKS * 2)
    b = (gbin // 2) % N_BLOCKS
    t = gbin & 1
    tile_in_bucket = j // P
    p = j % P
    # metadata in processing order
    tg = np.where(
        t == 0,
        TOFF[b] + tile_in_bucket,
        TOFF[b] + np.asarray(TL)[b] + tile_in_bucket,
    )
    dstcol[c, p, tg] = dcol
    normv[c, p, tg] = norm
    # gather index arrays (per-table tile order)
    idx16 = (msrc >> 1).astype(np.int16)
    Je = (EOFF[b] + tile_in_bucket) * P + p
    Jo = (OOFF[b] + tile_in_bucket) * P + p
    ev = t == 0
    idxe_flat[c[ev], Je[ev]] = idx16[ev]
    idxo_flat[c[~ev], Jo[~ev]] = idx16[~ev]

    # wrap: flat j -> (partition j%16, column j//16), replicated on 8 stripes
    def wrap(flat, ntiles):
        if ntiles == 0:
            return np.zeros((N_CORES, P, 0), np.int16)
        a = flat.reshape(N_CORES, ntiles * 8, 16).transpose(0, 2, 1)  # [8,16,cols]
        return np.ascontiguousarray(np.tile(a, (1, 8, 1)))  # [8,128,cols]

    return TL, TH, dstcol, normv, wrap(idxe_flat, sTL), wrap(idxo_flat, sTH)


def _chunks(TL, TH):
    """Group consecutive blocks into gather chunks where EACH table's tile
    count stays within one dma_gather call's limit."""
    out = []
    cur = []
    ne = no = 0
    for b in range(N_BLOCKS):
        if cur and (ne + TL[b] > GATHER_TILE_CAP or no + TH[b] > GATHER_TILE_CAP):
            out.append((cur, ne, no))
            cur, ne, no = [], 0, 0
        cur.append(b)
        ne += TL[b]
        no += TH[b]
    if cur:
        out.append((cur, ne, no))
    return out


def _build_program(TL, TH, generic_affine, bias_mean):
    import concourse.bass as bass
    import concourse.tile as tile
    from concourse import bacc as bacc_mod
    from concourse import mybir
    from contextlib import ExitStack

    f32 = mybir.dt.float32
    bf16 = mybir.dt.bfloat16
    cdt = bf16 if USE_BF16 else f32
    i16 = mybir.dt.int16
    Alu = mybir.AluOpType
    Act = mybir.ActivationFunctionType
    sTL, sTH = sum(TL), sum(TH)
    Ttot = sTL + sTH
    EOFF = np.concatenate([[0], np.cumsum(TL)])
    OOFF = np.concatenate([[0], np.cumsum(TH)])
    chunks = _chunks(TL, TH)
    max_ne = max(ch[1] for ch in chunks)
    max_no = max(ch[2] for ch in chunks)

    # fcon (f32) column layout: [dst | norm | bias | gamma? | beta?]
    FW = 2 * Ttot + WIDTH + (2 * WIDTH if generic_affine else 0)
    # bcon (cdt) column layout:  [wt_ext 2*(WIDTH+1) | iota (P)]
    BW = 2 * (WIDTH + 1) + P

    nc = bacc_mod.Bacc(None, target_bir_lowering=False, debug=False, num_swdge_queues=4)
    xe_d = nc.declare_dram_parameter("xe", [HALF, WIDTH], cdt, isOutput=False)
    xo_d = nc.declare_dram_parameter("xo", [HALF, WIDTH], cdt, isOutput=False)
    idxe_d = nc.declare_dram_parameter("idxe", [P, 8 * sTL], i16, isOutput=False)
    idxo_d = nc.declare_dram_parameter("idxo", [P, 8 * sTH], i16, isOutput=False)
    fcon_d = nc.declare_dram_parameter("fcon", [P, FW], f32, isOutput=False)
    bcon_d = nc.declare_dram_parameter("bcon", [P, BW], cdt, isOutput=False)
    out_d = nc.declare_dram_parameter("out", [NODES_PER_CORE, WIDTH], f32, isOutput=True)

    with tile.TileContext(nc) as tc:
        with ExitStack() as ctx:
            const = ctx.enter_context(tc.tile_pool(name="const", bufs=1))
            gpool = ctx.enter_context(tc.tile_pool(name="g", bufs=2))
            spool = ctx.enter_context(tc.tile_pool(name="s", bufs=6))
            apool = ctx.enter_context(tc.tile_pool(name="aggT", bufs=2))
            ypool = ctx.enter_context(tc.tile_pool(name="y", bufs=2))
            stat = ctx.enter_context(tc.tile_pool(name="stat", bufs=4))
            ppool = ctx.enter_context(tc.tile_pool(name="psA", bufs=2, space="PSUM"))
            opsum = ctx.enter_context(tc.tile_pool(name="psO", bufs=2, space="PSUM"))

            idxe_sb = const.tile([P, 8 * sTL], i16)
            nc.sync.dma_start(idxe_sb[:], idxe_d[:, :])
            idxo_sb = const.tile([P, 8 * sTH], i16)
            nc.sync.dma_start(idxo_sb[:], idxo_d[:, :])
            fcon_sb = const.tile([P, FW], f32)
            nc.sync.dma_start(fcon_sb[:], fcon_d[:, :])
            bcon_sb = const.tile([P, BW], cdt)
            nc.sync.dma_start(bcon_sb[:], bcon_d[:, :])
            eps_sb = const.tile([P, 1], f32)
            nc.vector.memset(eps_sb[:], LN_EPS)

            bias_sb = fcon_sb[:, 2 * Ttot : 2 * Ttot + WIDTH]
            if generic_affine:
                gamma_sb = fcon_sb[:, 2 * Ttot + WIDTH : 2 * Ttot + 2 * WIDTH]
                beta_sb = fcon_sb[:, 2 * Ttot + 2 * WIDTH : 2 * Ttot + 3 * WIDTH]
            wt_sb = bcon_sb[:, : 2 * (WIDTH + 1)]
            iota_sb = bcon_sb[:, 2 * (WIDTH + 1) : 2 * (WIDTH + 1) + P]
            bmean_sb = const.tile([P, 1], f32)
            nc.vector.memset(bmean_sb[:], bias_mean)

            qn = 0
            for blocks, ne, no in chunks:
                e0 = int(EOFF[blocks[0]])
                o0 = int(OOFF[blocks[0]])
                ge = go = None
                if ne:
                    ge = gpool.tile([P, ne, WIDTH], cdt, tag="ge")
                    nc.gpsimd.dma_gather(
                        ge[:],
                        xe_d[:, :],
                        idxe_sb[:, 8 * e0 : 8 * (e0 + ne)],
                        ne * P,
                        ne * P,
                        WIDTH,
                        queue_num=qn % 4,
                    )
                    qn += 1
                if no:
                    go = gpool.tile([P, no, WIDTH], cdt, tag="go")
                    nc.gpsimd.dma_gather(
                        go[:],
                        xo_d[:, :],
                        idxo_sb[:, 8 * o0 : 8 * (o0 + no)],
                        no * P,
                        no * P,
                        WIDTH,
                        queue_num=qn % 4,
                    )
                    qn += 1
                for b in blocks:
                    tg0 = int(
                        np.concatenate([[0], np.cumsum(np.asarray(TL) + np.asarray(TH))])[
                            b
                        ]
                    )
                    seq = [(ge, int(EOFF[b]) - e0 + t) for t in range(TL[b])] + [
                        (go, int(OOFF[b]) - o0 + t) for t in range(TH[b])
                    ]
                    nt = len(seq)
                    ps0 = ppool.tile([P, P], f32, tag="ps0")
                    ps1 = ppool.tile([P, P], f32, tag="ps1")
                    for k, (gt, col) in enumerate(seq):
                        tg = tg0 + k
                        s = spool.tile([P, P], cdt, tag="s")
                        nc.vector.tensor_scalar(
                            out=s[:],
                            in0=iota_sb,
                            scalar1=fcon_sb[:, tg : tg + 1],
                            scalar2=fcon_sb[:, Ttot + tg : Ttot + tg + 1],
                            op0=Alu.is_equal,
                            op1=Alu.mult,
                        )
                        nc.tensor.matmul(
                            out=ps0[:],
                            lhsT=gt[:, col, 0:P],
                            rhs=s[:],
                            start=(k == 0),
                            stop=(k == nt - 1),
                        )
                        nc.tensor.matmul(
                            out=ps1[:],
                            lhsT=gt[:, col, P:WIDTH],
                            rhs=s[:],
                            start=(k == 0),
                            stop=(k == nt - 1),
                        )
                    # aggT blocks [128 ch, 128 dst] -> SBUF (cast) for W-matmul
                    a0 = apool.tile([P, P], cdt, tag="a0")
                    nc.scalar.copy(a0[:], ps0[:])
                    a1 = apool.tile([P, P], cdt, tag="a1")
                    nc.scalar.copy(a1[:], ps1[:])
                    po = opsum.tile([P, WIDTH + 1], f32, tag="po")
                    nc.tensor.matmul(
                        out=po[:],
                        lhsT=a0[:],
                        rhs=wt_sb[:, : WIDTH + 1],
                        start=True,
                        stop=False,
                    )
                    nc.tensor.matmul(
                        out=po[:],
                        lhsT=a1[:],
                        rhs=wt_sb[:, WIDTH + 1 :],
                        start=False,
                        stop=True,
                    )
                    # ---- epilogue: y = po + bias; LayerNorm; ReLU ----
                    y = ypool.tile([P, WIDTH], f32, tag="y")
                    # NOTE: tensor_tensor_reduce hard-crashes TRN2 here; plain
                    # add, with the row-sum coming free from the W-matmul's
                    # extra weight column (po[:, WIDTH]).
                    nc.vector.tensor_tensor(
                        out=y[:], in0=po[:, :WIDTH], in1=bias_sb, op=Alu.add
                    )
                    sq = ypool.tile([P, WIDTH], f32, tag="sq")
                    ssq = stat.tile([P, 1], f32, tag="ssq")
                    nc.scalar.activation(
                        out=sq[:], in_=y[:], func=Act.Square, accum_out=ssq[:]
                    )
                    mu = stat.tile([P, 1], f32, tag="mu")
                    nc.scalar.activation(
                        out=mu[:],
                        in_=po[:, WIDTH : WIDTH + 1],
                        func=Act.Identity,
                        scale=1.0 / WIDTH,
                        bias=bmean_sb[:, :1],
                    )
                    m2 = stat.tile([P, 1], f32, tag="m2")
                    nc.scalar.square(m2[:], mu[:])
                    var = stat.tile([P, 1], f32, tag="var")
                    nc.vector.tensor_scalar(
                        out=var[:],
                        in0=ssq[:],
                        scalar1=1.0 / WIDTH,
                        scalar2=m2[:, :1],
                        op0=Alu.mult,
                        op1=Alu.subtract,
                    )
                    sd = stat.tile([P, 1], f32, tag="sd")
                    nc.scalar.activation(
                        out=sd[:], in_=var[:], func=Act.Sqrt, bias=eps_sb[:, :1]
                    )
                    rstd = stat.tile([P, 1], f32, tag="rstd")
                    nc.vector.reciprocal(rstd[:], sd[:])
                    t1 = ypool.tile([P, WIDTH], f32, tag="t1")
                    nc.vector.tensor_scalar(
                        out=t1[:],
                        in0=y[:],
                        scalar1=mu[:, :1],
                        scalar2=rstd[:, :1],
                        op0=Alu.subtract,
                        op1=Alu.mult,
                    )
                    if generic_affine:
                        t2 = ypool.tile([P, WIDTH], f32, tag="t2")
                        nc.vector.tensor_tensor(
                            out=t2[:], in0=t1[:], in1=gamma_sb, op=Alu.mult
                        )
                        t3 = ypool.tile([P, WIDTH], f32, tag="t3")
                        nc.vector.tensor_tensor(
                            out=t3[:], in0=t2[:], in1=beta_sb, op=Alu.add
                        )
                        t1 = t3
                    yo = ypool.tile([P, WIDTH], f32, tag="yo")
                    nc.scalar.activation(out=yo[:], in_=t1[:], func=Act.Relu)
                    rows = min(P, NODES_PER_CORE - b * P)
                    nc.sync.dma_start(out_d[b * P : b * P + rows, :], yo[:rows, :])
    return nc


def _pack_inputs(TL, TH, dstcol, normv, idxe, idxo, x, W, bias, gamma, beta, generic_affine):
    cnp = ml_dtypes.bfloat16 if USE_BF16 else np.float32
    Ttot = sum(TL) + sum(TH)

    xc = x.astype(cnp)
    xe = np.ascontiguousarray(xc[0::2])
    xo = np.ascontiguousarray(xc[1::2])
    WT32 = W.T.astype(np.float32)  # [in, out]
    rs = WT32.sum(axis=1, keepdims=True)  # [256, 1] row sums
    WTe = np.concatenate([WT32, rs], axis=1).astype(cnp)  # [256, 257]
    wt = np.concatenate([WTe[:P], WTe[P:]], axis=1)  # [128, 514]
    iota = np.tile(np.arange(P), (P, 1)).astype(cnp)
    bcon = np.ascontiguousarray(np.concatenate([wt, iota], axis=1))

    biasb = np.tile(bias.astype(np.float32)[None, :], (P, 1))
    fparts = [None, None, biasb]
    if generic_affine:
        fparts.append(np.tile(gamma.astype(np.float32)[None, :], (P, 1)))
        fparts.append(np.tile(beta.astype(np.float32)[None, :], (P, 1)))

    in_maps = []
    for c in range(N_CORES):
        fparts[0] = dstcol[c]
        fparts[1] = normv[c]
        fcon = np.ascontiguousarray(np.concatenate(fparts, axis=1, dtype=np.float32))
        in_maps.append(
            {
                "xe": xe,
                "xo": xo,
                "idxe": np.ascontiguousarray(idxe[c]),
                "idxo": np.ascontiguousarray(idxo[c]),
                "fcon": fcon,
                "bcon": bcon,
            }
        )
    return in_maps


_PROGRAM_CACHE = {}


def kernel(x, edge_index, W, b, gamma, beta, _run_kwargs=None):
    from concourse.bass_utils import run_bass_kernel_spmd

    x = np.asarray(x)
    W = np.asarray(W)
    bias = np.asarray(b)
    gamma = np.asarray(gamma)
    beta = np.asarray(beta)

    TL, TH, dstcol, normv, idxe, idxo = _preprocess(edge_index)
    generic_affine = not (np.all(gamma == 1.0) and np.all(beta == 0.0))

    bias_mean = float(bias.astype(np.float64).mean())
    key = (tuple(TL), tuple(TH), generic_affine, bias_mean)
    if key not in _PROGRAM_CACHE:
        nc = _build_program(TL, TH, generic_affine, bias_mean)
        nc.finalize()
        _PROGRAM_CACHE[key] = nc
    nc = _PROGRAM_CACHE[key]

    in_maps = _pack_inputs(
        TL, TH, dstcol, normv, idxe, idxo, x, W, bias, gamma, beta, generic_affine
    )

    kwargs = dict(_run_kwargs or {})
    kwargs.pop("_result", None)
    rr = run_bass_kernel_spmd(nc, in_maps, list(range(N_CORES)), **kwargs)
    out = np.concatenate([rr.results[c]["out"] for c in range(N_CORES)], axis=0)
    if _run_kwargs is not None:
        _run_kwargs["_result"] = rr
    return np.ascontiguousarray(out.astype(np.float32))



# revision 4
# speedup vs baseline: 1.3337x; 1.3337x over previous
"""GCN block (GCNConv + LayerNorm + ReLU) on 8 Trainium2 NeuronCores.

Strategy (v2 — "shard nodes / partition edges by destination" hint):
  - out = LN((A_norm @ x) @ W^T + b): aggregate raw features first (A_norm
    commutes with the linear map), so the random gather runs on node-major
    x and no transposes are needed anywhere.
  - Destination nodes are sharded contiguously across the 8 cores
    (6250 rows each); each core processes the edges that point into its
    shard.  The gather tables are PRE-SCALED by dinv[src] on the host, so
    the per-128-dst-block selection matrices S are PURE one-hot and are
    precomputed on the host as an fp8 stream (1.0 is exact in fp8e4m3),
    streamed from DRAM via HWDGE — no DVE build work on-chip at all.
  - Self-loop messages never go through dma_gather: the core's own
    (dinv-scaled) x-slice sits resident in SBUF and enters each block's
    PSUM accumulation through an identity-rhs matmul (a transpose).
  - The remaining dst-side dinv scaling and the bias are folded into the
    W-matmul: po = aggT^T @ [W^T | rowsum] + (1/dinv_dst) * [b | sum(b)]
    via a K=1 matmul with lhsT = sqrt(deg) per dst, so y = dinv_dst * po
    exactly.  LayerNorm statistics then reduce to per-partition scalars
    computed from po's row-sum column and one Square+accum pass, and the
    final normalize+ReLU collapses into a single ACT activation with
    per-partition scale/bias.
"""

import math
import sys

sys.path.insert(0, "/opt/trn_rl_repo")

import numpy as np
import ml_dtypes

N_NODES = 50000
WIDTH = 256
N_CORES = 8
NODES_PER_CORE = N_NODES // N_CORES  # 6250
P = 128
N_BLOCKS = math.ceil(NODES_PER_CORE / P)  # 49 (last block has 106 rows)
LN_EPS = 1e-5
HALF = N_NODES // 2  # rows per gather table

GATHER_TILE_CAP = 8  # max tiles (128 idxs each) per dma_gather call (HW limit 1024)


def _preprocess(edge_index):
    """Bucket non-self-loop messages by (core, dst-block, src-parity table),
    pad each bucket to whole 128-edge tiles.

    Processing tile order: per block, even-table tiles then odd-table tiles.
    Gather order: even tiles of all blocks concatenated (ditto odd).
    Returns (TL, TH, deg, idxe[8,128,8*sTL] i16, idxo[...] i16,
             scon[8,128,Ttot*128] fp8 one-hot).
    """
    src = np.asarray(edge_index[0]).astype(np.int64)
    dst = np.asarray(edge_index[1]).astype(np.int64)

    deg = (np.bincount(dst, minlength=N_NODES) + 1).astype(np.float64)  # + self loop

    core = dst // NODES_PER_CORE
    r = dst % NODES_PER_CORE
    blk = np.minimum(r // P, N_BLOCKS - 1)
    dcol = r - blk * P
    tab = src & 1
    gbin = (core * N_BLOCKS + blk) * 2 + tab

    order = np.argsort(gbin, kind="stable")
    src, dcol, gbin = src[order], dcol[order], gbin[order]

    cnt = np.bincount(gbin, minlength=N_CORES * N_BLOCKS * 2).reshape(
        N_CORES, N_BLOCKS, 2
    )
    TL = [int(math.ceil(int(cnt[:, b, 0].max()) / P)) for b in range(N_BLOCKS)]
    TH = [int(math.ceil(int(cnt[:, b, 1].max()) / P)) for b in range(N_BLOCKS)]
    sTL, sTH = sum(TL), sum(TH)
    Ttot = sTL + sTH
    EOFF = np.concatenate([[0], np.cumsum(TL)])  # even gather order
    OOFF = np.concatenate([[0], np.cumsum(TH)])  # odd gather order
    TOFF = np.concatenate([[0], np.cumsum(np.asarray(TL) + np.asarray(TH))])

    idxe_flat = np.zeros((N_CORES, sTL * P), np.int16)
    idxo_flat = np.zeros((N_CORES, sTH * P), np.int16)
    scon = np.zeros((N_CORES, P, Ttot * P), ml_dtypes.float8_e4m3)

    starts = np.concatenate([[0], np.cumsum(cnt.ravel())])[:-1]
    j = np.arange(len(gbin)) - starts[gbin]  # index within bucket
    c = gbin // (N_BLOCKS * 2)
    b = (gbin // 2) % N_BLOCKS
    t = gbin & 1
    tile_in_bucket = j // P
    p = j % P
    # processing-order tile id (even tiles of block b, then odd tiles)
    tg = np.where(
        t == 0,
        TOFF[b] + tile_in_bucket,
        TOFF[b] + np.asarray(TL)[b] + tile_in_bucket,
    )
    scon[c, p, tg * P + dcol] = 1.0
    # gather index arrays (per-table tile order)
    idx16 = (src >> 1).astype(np.int16)
    Je = (EOFF[b] + tile_in_bucket) * P + p
    Jo = (OOFF[b] + tile_in_bucket) * P + p
    ev = t == 0
    idxe_flat[c[ev], Je[ev]] = idx16[ev]
    idxo_flat[c[~ev], Jo[~ev]] = idx16[~ev]

    # wrap: flat j -> (partition j%16, column j//16), replicated on 8 stripes
    def wrap(flat, ntiles):
        if ntiles == 0:
            return np.zeros((N_CORES, P, 0), np.int16)
        a = flat.reshape(N_CORES, ntiles * 8, 16).transpose(0, 2, 1)  # [8,16,cols]
        return np.ascontiguousarray(np.tile(a, (1, 8, 1)))  # [8,128,cols]

    return TL, TH, deg, wrap(idxe_flat, sTL), wrap(idxo_flat, sTH), scon


def _chunks(TL, TH):
    """Group consecutive blocks into gather chunks where EACH table's tile
    count stays within one dma_gather call's limit."""
    out = []
    cur = []
    ne = no = 0
    for b in range(N_BLOCKS):
        if cur and (ne + TL[b] > GATHER_TILE_CAP or no + TH[b] > GATHER_TILE_CAP):
            out.append((cur, ne, no))
            cur, ne, no = [], 0, 0
        cur.append(b)
        ne += TL[b]
        no += TH[b]
    if cur:
        out.append((cur, ne, no))
    return out


def _build_program(TL, TH, generic_affine):
    import concourse.bass as bass
    import concourse.tile as tile
    from concourse import bacc as bacc_mod
    from concourse import mybir
    from contextlib import ExitStack

    f32 = mybir.dt.float32
    bf16 = mybir.dt.bfloat16
    fp8 = mybir.dt.float8e4
    i16 = mybir.dt.int16
    Alu = mybir.AluOpType
    Act = mybir.ActivationFunctionType
    sTL, sTH = sum(TL), sum(TH)
    Ttot = sTL + sTH
    EOFF = np.concatenate([[0], np.cumsum(TL)])
    OOFF = np.concatenate([[0], np.cumsum(TH)])
    TOFF = np.concatenate([[0], np.cumsum(np.asarray(TL) + np.asarray(TH))])
    chunks = _chunks(TL, TH)

    W2 = WIDTH  # 256; per-block xself column stride
    # wcon (bf16) column layout: [wt 2*(WIDTH+1) | identity (P)]
    BW = 2 * (WIDTH + 1) + P
    # vrow (bf16) layout on one partition: [brow (WIDTH+1) | sqrt(deg) per dst]
    VW = (WIDTH + 1) + N_BLOCKS * P
    NW = WIDTH + 1  # 257

    nc = bacc_mod.Bacc(None, target_bir_lowering=False, debug=False, num_swdge_queues=4)
    xe_d = nc.declare_dram_parameter("xe", [HALF, WIDTH], bf16, isOutput=False)
    xo_d = nc.declare_dram_parameter("xo", [HALF, WIDTH], bf16, isOutput=False)
    idxe_d = nc.declare_dram_parameter("idxe", [P, 8 * sTL], i16, isOutput=False)
    idxo_d = nc.declare_dram_parameter("idxo", [P, 8 * sTH], i16, isOutput=False)
    scon_d = nc.declare_dram_parameter("scon", [P, Ttot * P], fp8, isOutput=False)
    xself_d = nc.declare_dram_parameter("xself", [P, N_BLOCKS * W2], bf16, isOutput=False)
    fcon_d = nc.declare_dram_parameter("fcon", [P, N_BLOCKS], f32, isOutput=False)
    wcon_d = nc.declare_dram_parameter("wcon", [P, BW], bf16, isOutput=False)
    vrow_d = nc.declare_dram_parameter("vrow", [1, VW], bf16, isOutput=False)
    if generic_affine:
        gb_d = nc.declare_dram_parameter("gbcon", [P, 2 * WIDTH], f32, isOutput=False)
    out_d = nc.declare_dram_parameter("out", [NODES_PER_CORE, WIDTH], f32, isOutput=True)

    with tile.TileContext(nc) as tc:
        with ExitStack() as ctx:
            const = ctx.enter_context(tc.tile_pool(name="const", bufs=1))
            gpool = ctx.enter_context(tc.tile_pool(name="g", bufs=4))
            spool = ctx.enter_context(tc.tile_pool(name="s", bufs=3))
            apool = ctx.enter_context(tc.tile_pool(name="aggT", bufs=3))
            ypool = ctx.enter_context(tc.tile_pool(name="y", bufs=2))
            sqpool = ctx.enter_context(tc.tile_pool(name="sq", bufs=2))
            stat = ctx.enter_context(tc.tile_pool(name="stat", bufs=6))
            ppool = ctx.enter_context(tc.tile_pool(name="psA", bufs=2, space="PSUM"))
            opsum = ctx.enter_context(tc.tile_pool(name="psO", bufs=2, space="PSUM"))

            idxe_sb = const.tile([P, 8 * sTL], i16)
            nc.sync.dma_start(idxe_sb[:], idxe_d[:, :])
            idxo_sb = const.tile([P, 8 * sTH], i16)
            nc.sync.dma_start(idxo_sb[:], idxo_d[:, :])
            xself_sb = const.tile([P, N_BLOCKS * W2], bf16)
            nc.sync.dma_start(xself_sb[:], xself_d[:, :])
            fcon_sb = const.tile([P, N_BLOCKS], f32)
            nc.sync.dma_start(fcon_sb[:], fcon_d[:, :])
            wcon_sb = const.tile([P, BW], bf16)
            nc.sync.dma_start(wcon_sb[:], wcon_d[:, :])
            vrow_sb = const.tile([1, VW], bf16)
            nc.sync.dma_start(vrow_sb[:], vrow_d[:, :])
            if generic_affine:
                gb_sb = const.tile([P, 2 * WIDTH], f32)
                nc.sync.dma_start(gb_sb[:], gb_d[:, :])

            wt_sb = wcon_sb[:, : 2 * NW]
            ident_sb = wcon_sb[:, 2 * NW : 2 * NW + P]

            qn = 0
            for blocks, ne, no in chunks:
                e0 = int(EOFF[blocks[0]])
                o0 = int(OOFF[blocks[0]])
                t0 = int(TOFF[blocks[0]])
                nt = sum(TL[b] + TH[b] for b in blocks)
                ge = go = None
                if ne:
                    ge = gpool.tile([P, ne, WIDTH], bf16, tag="ge")
                    nc.gpsimd.dma_gather(
                        ge[:],
                        xe_d[:, :],
                        idxe_sb[:, 8 * e0 : 8 * (e0 + ne)],
                        ne * P,
                        ne * P,
                        WIDTH,
                        queue_num=qn % 4,
                    )
                    qn += 1
                if no:
                    go = gpool.tile([P, no, WIDTH], bf16, tag="go")
                    nc.gpsimd.dma_gather(
                        go[:],
                        xo_d[:, :],
                        idxo_sb[:, 8 * o0 : 8 * (o0 + no)],
                        no * P,
                        no * P,
                        WIDTH,
                        queue_num=qn % 4,
                    )
                    qn += 1
                sc = spool.tile([P, nt * P], fp8, tag="sc")
                nc.sync.dma_start(sc[:], scon_d[:, t0 * P : (t0 + nt) * P])
                for b in blocks:
                    tl = int(TOFF[b]) - t0
                    seq = [(ge, int(EOFF[b]) - e0 + t) for t in range(TL[b])] + [
                        (go, int(OOFF[b]) - o0 + t) for t in range(TH[b])
                    ]
                    ntb = len(seq)
                    # NOTE: matmul start=True resets the whole PSUM bank, so
                    # the two ch-half accumulation groups need separate tiles
                    # (separate banks), not column windows of one tile.
                    ps0 = ppool.tile([P, P], f32, tag="ps0")
                    ps1 = ppool.tile([P, P], f32, tag="ps1")
                    # self-loop contribution: aggT += (dinv*x_self)^T via identity
                    nc.tensor.matmul(
                        out=ps0[:],
                        lhsT=xself_sb[:, b * W2 : b * W2 + P],
                        rhs=ident_sb,
                        start=True,
                        stop=False,
                    )
                    nc.tensor.matmul(
                        out=ps1[:],
                        lhsT=xself_sb[:, b * W2 + P : (b + 1) * W2],
                        rhs=ident_sb,
                        start=True,
                        stop=False,
                    )
                    for k, (gt, col) in enumerate(seq):
                        s_ap = sc[:, (tl + k) * P : (tl + k + 1) * P]
                        last = k == ntb - 1
                        nc.tensor.matmul(
                            out=ps0[:],
                            lhsT=gt[:, col, 0:P],
                            rhs=s_ap,
                            start=False,
                            stop=last,
                        )
                        nc.tensor.matmul(
                            out=ps1[:],
                            lhsT=gt[:, col, P:WIDTH],
                            rhs=s_ap,
                            start=False,
                            stop=last,
                        )
                    # aggT [2x 128ch, 128 dst] -> SBUF (cast) for the W-matmul
                    a = apool.tile([P, 2 * P], bf16, tag="a")
                    nc.vector.tensor_scalar_mul(a[:, 0:P], ps0[:], 1.0)
                    nc.scalar.copy(a[:, P : 2 * P], ps1[:])
                    po = opsum.tile([P, NW], f32, tag="po")
                    nc.tensor.matmul(
                        out=po[:],
                        lhsT=a[:, 0:P],
                        rhs=wt_sb[:, 0:NW],
                        start=True,
                        stop=False,
                    )
                    nc.tensor.matmul(
                        out=po[:],
                        lhsT=a[:, P : 2 * P],
                        rhs=wt_sb[:, NW : 2 * NW],
                        start=False,
                        stop=False,
                    )
                    # bias row scaled by sqrt(deg) so y = dinv_dst * po exactly
                    nc.tensor.matmul(
                        out=po[:],
                        lhsT=vrow_sb[:, NW + b * P : NW + (b + 1) * P],
                        rhs=vrow_sb[:, 0:NW],
                        start=False,
                        stop=True,
                    )
                    # ---- epilogue: LN + ReLU on y = dinv*po, all per-partition ----
                    sq = sqpool.tile([P, WIDTH], f32, tag="sq")
                    ssq = stat.tile([P, 1], f32, tag="ssq")
                    nc.scalar.activation(
                        out=sq[:], in_=po[:, :WIDTH], func=Act.Square, accum_out=ssq[:]
                    )
                    m2 = stat.tile([P, 1], f32, tag="m2")
                    nc.scalar.activation(
                        out=m2[:],
                        in_=po[:, WIDTH : WIDTH + 1],
                        func=Act.Square,
                        scale=1.0 / WIDTH,
                    )
                    rv = stat.tile([P, 1], f32, tag="rv")
                    nc.vector.tensor_scalar(
                        out=rv[:],
                        in0=ssq[:],
                        scalar1=1.0 / WIDTH,
                        scalar2=m2[:, :1],
                        op0=Alu.mult,
                        op1=Alu.subtract,
                    )
                    sd = stat.tile([P, 1], f32, tag="sd")
                    nc.scalar.activation(
                        out=sd[:], in_=rv[:], func=Act.Sqrt, bias=fcon_sb[:, b : b + 1]
                    )
                    rstd = stat.tile([P, 1], f32, tag="rstd")
                    nc.vector.reciprocal(rstd[:], sd[:])
                    nb = stat.tile([P, 1], f32, tag="nb")
                    nc.vector.tensor_scalar(
                        out=nb[:],
                        in0=po[:, WIDTH : WIDTH + 1],
                        scalar1=-1.0 / WIDTH,
                        scalar2=rstd[:, :1],
                        op0=Alu.mult,
                        op1=Alu.mult,
                    )
                    yo = ypool.tile([P, WIDTH], f32, tag="yo")
                    if not generic_affine:
                        nc.scalar.activation(
                            out=yo[:],
                            in_=po[:, :WIDTH],
                            func=Act.Relu,
                            scale=rstd[:, :1],
                            bias=nb[:, :1],
                        )
                    else:
                        t1 = ypool.tile([P, WIDTH], f32, tag="t1")
                        nc.scalar.activation(
                            out=t1[:],
                            in_=po[:, :WIDTH],
                            func=Act.Identity,
                            scale=rstd[:, :1],
                            bias=nb[:, :1],
                        )
                        t2 = ypool.tile([P, WIDTH], f32, tag="t2")
                        nc.vector.tensor_tensor(
                            out=t2[:], in0=t1[:], in1=gb_sb[:, :WIDTH], op=Alu.mult
                        )
                        t3 = ypool.tile([P, WIDTH], f32, tag="t3")
                        nc.vector.tensor_tensor(
                            out=t3[:], in0=t2[:], in1=gb_sb[:, WIDTH:], op=Alu.add
                        )
                        nc.scalar.activation(out=yo[:], in_=t3[:], func=Act.Relu)
                    rows = min(P, NODES_PER_CORE - b * P)
                    nc.sync.dma_start(out_d[b * P : b * P + rows, :], yo[:rows, :])
    return nc


def _pack_inputs(TL, TH, deg, idxe, idxo, scon, x, W, bias, gamma, beta, generic_affine):
    bfnp = ml_dtypes.bfloat16

    dinv = (1.0 / np.sqrt(deg)).astype(np.float64)
    xs = (np.asarray(x, np.float64) * dinv[:, None]).astype(bfnp)  # dinv-prescaled
    xe = np.ascontiguousarray(xs[0::2])
    xo = np.ascontiguousarray(xs[1::2])

    # xself: per core, [128, 49*256]: xself[p, b*256+ch] = xs[c*6250+b*128+p, ch]
    xself_all = np.zeros((N_CORES, N_BLOCKS, P, WIDTH), bfnp)
    for c in range(N_CORES):
        sl = xs[c * NODES_PER_CORE : (c + 1) * NODES_PER_CORE]
        flat = np.zeros((N_BLOCKS * P, WIDTH), bfnp)
        flat[: NODES_PER_CORE] = sl
        xself_all[c] = flat.reshape(N_BLOCKS, P, WIDTH)
    xself_all = np.ascontiguousarray(
        xself_all.transpose(0, 2, 1, 3).reshape(N_CORES, P, N_BLOCKS * WIDTH)
    )

    # fcon: eps*deg per (partition, block); pads -> deg 1
    degp = np.ones((N_CORES, N_BLOCKS * P), np.float64)
    for c in range(N_CORES):
        degp[c, :NODES_PER_CORE] = deg[c * NODES_PER_CORE : (c + 1) * NODES_PER_CORE]
    epsdeg = (LN_EPS * degp).astype(np.float32).reshape(N_CORES, N_BLOCKS, P)
    epsdeg = np.ascontiguousarray(epsdeg.transpose(0, 2, 1))  # [8, 128, 49]

    WT32 = np.asarray(W, np.float64).T  # [in, out]
    rs = WT32.sum(axis=1, keepdims=True)
    WTe = np.concatenate([WT32, rs], axis=1).astype(bfnp)  # [256, 257]
    wt = np.concatenate([WTe[:P], WTe[P:]], axis=1)  # [128, 514]
    ident = np.eye(P, dtype=bfnp)
    wcon = np.ascontiguousarray(np.concatenate([wt, ident], axis=1))

    b64 = np.asarray(bias, np.float64)
    brow = np.concatenate([b64, [b64.sum()]])  # [257]
    sdeg = np.sqrt(degp)  # [8, N_BLOCKS*P]
    vrow_all = np.concatenate(
        [np.tile(brow[None, :], (N_CORES, 1)), sdeg], axis=1
    ).astype(bfnp)  # [8, 257 + 6272]

    in_maps = []
    for c in range(N_CORES):
        m = {
            "xe": xe,
            "xo": xo,
            "idxe": np.ascontiguousarray(idxe[c]),
            "idxo": np.ascontiguousarray(idxo[c]),
            "scon": np.ascontiguousarray(scon[c]),
            "xself": xself_all[c],
            "fcon": epsdeg[c],
            "wcon": wcon,
            "vrow": vrow_all[c : c + 1],
        }
        if generic_affine:
            gb = np.concatenate(
                [
                    np.tile(np.asarray(gamma, np.float32)[None, :], (P, 1)),
                    np.tile(np.asarray(beta, np.float32)[None, :], (P, 1)),
                ],
                axis=1,
            )
            m["gbcon"] = np.ascontiguousarray(gb)
        in_maps.append(m)
    return in_maps


_PROGRAM_CACHE = {}


def kernel(x, edge_index, W, b, gamma, beta, _run_kwargs=None):
    from concourse.bass_utils import run_bass_kernel_spmd

    x = np.asarray(x)
    W = np.asarray(W)
    bias = np.asarray(b)
    gamma = np.asarray(gamma)
    beta = np.asarray(beta)

    TL, TH, deg, idxe, idxo, scon = _preprocess(edge_index)
    generic_affine = not (np.all(gamma == 1.0) and np.all(beta == 0.0))

    key = (tuple(TL), tuple(TH), generic_affine)
    if key not in _PROGRAM_CACHE:
        nc = _build_program(TL, TH, generic_affine)
        nc.finalize()
        _PROGRAM_CACHE[key] = nc
    nc = _PROGRAM_CACHE[key]

    in_maps = _pack_inputs(
        TL, TH, deg, idxe, idxo, scon, x, W, bias, gamma, beta, generic_affine
    )

    kwargs = dict(_run_kwargs or {})
    kwargs.pop("_result", None)
    rr = run_bass_kernel_spmd(nc, in_maps, list(range(N_CORES)), **kwargs)
    out = np.concatenate([rr.results[c]["out"] for c in range(N_CORES)], axis=0)
    if _run_kwargs is not None:
        _run_kwargs["_result"] = rr
    return np.ascontiguousarray(out.astype(np.float32))


# revision 7
# speedup vs baseline: 1.3570x; 1.0175x over previous
"""GCN block (GCNConv + LayerNorm + ReLU) on 8 Trainium2 NeuronCores.

Strategy (v2 — "shard nodes / partition edges by destination" hint):
  - out = LN((A_norm @ x) @ W^T + b): aggregate raw features first (A_norm
    commutes with the linear map), so the random gather runs on node-major
    x and no transposes are needed anywhere.
  - Destination nodes are sharded contiguously across the 8 cores
    (6250 rows each); each core processes the edges that point into its
    shard.  The gather tables are PRE-SCALED by dinv[src] on the host, so
    the per-128-dst-block selection matrices S are PURE one-hot and are
    precomputed on the host as an fp8 stream (1.0 is exact in fp8e4m3),
    streamed from DRAM via HWDGE — no DVE build work on-chip at all.
  - Self-loop messages never go through dma_gather: the core's own
    (dinv-scaled) x-slice sits resident in SBUF and enters each block's
    PSUM accumulation through an identity-rhs matmul (a transpose).
  - The remaining dst-side dinv scaling and the bias are folded into the
    W-matmul: po = aggT^T @ [W^T | rowsum] + (1/dinv_dst) * [b | sum(b)]
    via a K=1 matmul with lhsT = sqrt(deg) per dst, so y = dinv_dst * po
    exactly.  LayerNorm statistics then reduce to per-partition scalars
    computed from po's row-sum column and one Square+accum pass, and the
    final normalize+ReLU collapses into a single ACT activation with
    per-partition scale/bias.
"""

import math
import sys

sys.path.insert(0, "/opt/trn_rl_repo")

import numpy as np
import ml_dtypes

N_NODES = 50000
WIDTH = 256
N_CORES = 8
NODES_PER_CORE = N_NODES // N_CORES  # 6250
P = 128
N_BLOCKS = math.ceil(NODES_PER_CORE / P)  # 49 (last block has 106 rows)
LN_EPS = 1e-5
HALF = N_NODES // 2  # rows per gather table

GATHER_TILE_CAP = 8  # max tiles (128 idxs each) per dma_gather call (HW limit 1024)


def _preprocess(edge_index):
    """Bucket non-self-loop messages by (core, dst-block, src-parity table),
    pad each bucket to whole 128-edge tiles.

    Processing tile order: per block, even-table tiles then odd-table tiles.
    Gather order: even tiles of all blocks concatenated (ditto odd).
    Returns (TL, TH, deg, idxe[8,128,8*sTL] i16, idxo[...] i16,
             scon[8,128,Ttot*128] fp8 one-hot).
    """
    src = np.asarray(edge_index[0]).astype(np.int64)
    dst = np.asarray(edge_index[1]).astype(np.int64)

    deg = (np.bincount(dst, minlength=N_NODES) + 1).astype(np.float64)  # + self loop

    core = dst // NODES_PER_CORE
    r = dst % NODES_PER_CORE
    blk = np.minimum(r // P, N_BLOCKS - 1)
    dcol = r - blk * P
    tab = src & 1
    gbin = (core * N_BLOCKS + blk) * 2 + tab

    order = np.argsort(gbin, kind="stable")
    src, dcol, gbin = src[order], dcol[order], gbin[order]

    cnt = np.bincount(gbin, minlength=N_CORES * N_BLOCKS * 2).reshape(
        N_CORES, N_BLOCKS, 2
    )
    TL = [int(math.ceil(int(cnt[:, b, 0].max()) / P)) for b in range(N_BLOCKS)]
    TH = [int(math.ceil(int(cnt[:, b, 1].max()) / P)) for b in range(N_BLOCKS)]
    sTL, sTH = sum(TL), sum(TH)
    Ttot = sTL + sTH
    EOFF = np.concatenate([[0], np.cumsum(TL)])  # even gather order
    OOFF = np.concatenate([[0], np.cumsum(TH)])  # odd gather order
    TOFF = np.concatenate([[0], np.cumsum(np.asarray(TL) + np.asarray(TH))])

    idxe_flat = np.zeros((N_CORES, sTL * P), np.int16)
    idxo_flat = np.zeros((N_CORES, sTH * P), np.int16)
    scon = np.zeros((N_CORES, P, Ttot * P), ml_dtypes.float8_e4m3)

    starts = np.concatenate([[0], np.cumsum(cnt.ravel())])[:-1]
    j = np.arange(len(gbin)) - starts[gbin]  # index within bucket
    c = gbin // (N_BLOCKS * 2)
    b = (gbin // 2) % N_BLOCKS
    t = gbin & 1
    tile_in_bucket = j // P
    p = j % P
    # processing-order tile id (even tiles of block b, then odd tiles)
    tg = np.where(
        t == 0,
        TOFF[b] + tile_in_bucket,
        TOFF[b] + np.asarray(TL)[b] + tile_in_bucket,
    )
    scon[c, p, tg * P + dcol] = 1.0
    # gather index arrays (per-table tile order)
    idx16 = (src >> 1).astype(np.int16)
    Je = (EOFF[b] + tile_in_bucket) * P + p
    Jo = (OOFF[b] + tile_in_bucket) * P + p
    ev = t == 0
    idxe_flat[c[ev], Je[ev]] = idx16[ev]
    idxo_flat[c[~ev], Jo[~ev]] = idx16[~ev]

    # wrap: flat j -> (partition j%16, column j//16), replicated on 8 stripes
    def wrap(flat, ntiles):
        if ntiles == 0:
            return np.zeros((N_CORES, P, 0), np.int16)
        a = flat.reshape(N_CORES, ntiles * 8, 16).transpose(0, 2, 1)  # [8,16,cols]
        return np.ascontiguousarray(np.tile(a, (1, 8, 1)))  # [8,128,cols]

    return TL, TH, deg, wrap(idxe_flat, sTL), wrap(idxo_flat, sTH), scon


def _chunks(TL, TH):
    """Group consecutive blocks into gather chunks where EACH table's tile
    count stays within one dma_gather call's limit."""
    out = []
    cur = []
    ne = no = 0
    for b in range(N_BLOCKS):
        if cur and (ne + TL[b] > GATHER_TILE_CAP or no + TH[b] > GATHER_TILE_CAP):
            out.append((cur, ne, no))
            cur, ne, no = [], 0, 0
        cur.append(b)
        ne += TL[b]
        no += TH[b]
    if cur:
        out.append((cur, ne, no))
    return out


def _build_program(TL, TH, generic_affine):
    import concourse.bass as bass
    import concourse.tile as tile
    from concourse import bacc as bacc_mod
    from concourse import mybir
    from contextlib import ExitStack

    f32 = mybir.dt.float32
    bf16 = mybir.dt.bfloat16
    fp8 = mybir.dt.float8e4
    i16 = mybir.dt.int16
    Alu = mybir.AluOpType
    Act = mybir.ActivationFunctionType
    sTL, sTH = sum(TL), sum(TH)
    Ttot = sTL + sTH
    EOFF = np.concatenate([[0], np.cumsum(TL)])
    OOFF = np.concatenate([[0], np.cumsum(TH)])
    TOFF = np.concatenate([[0], np.cumsum(np.asarray(TL) + np.asarray(TH))])
    chunks = _chunks(TL, TH)

    W2 = WIDTH  # 256; per-block xself column stride
    # wcon (bf16) column layout: [wt 2*(WIDTH+1) | identity (P)]
    BW = 2 * (WIDTH + 1) + P
    # vrow (bf16) layout on one partition: [brow (WIDTH+1) | sqrt(deg) per dst]
    VW = (WIDTH + 1) + N_BLOCKS * P
    NW = WIDTH + 1  # 257

    nc = bacc_mod.Bacc(None, target_bir_lowering=False, debug=False, num_swdge_queues=4)
    xe_d = nc.declare_dram_parameter("xe", [HALF, WIDTH], bf16, isOutput=False)
    xo_d = nc.declare_dram_parameter("xo", [HALF, WIDTH], bf16, isOutput=False)
    idxe_d = nc.declare_dram_parameter("idxe", [P, 8 * sTL], i16, isOutput=False)
    idxo_d = nc.declare_dram_parameter("idxo", [P, 8 * sTH], i16, isOutput=False)
    scon_d = nc.declare_dram_parameter("scon", [P, Ttot * P], fp8, isOutput=False)
    xself_d = nc.declare_dram_parameter("xself", [P, N_BLOCKS * W2], bf16, isOutput=False)
    fcon_d = nc.declare_dram_parameter("fcon", [P, N_BLOCKS], f32, isOutput=False)
    wcon_d = nc.declare_dram_parameter("wcon", [P, BW], bf16, isOutput=False)
    vrow_d = nc.declare_dram_parameter("vrow", [1, VW], bf16, isOutput=False)
    if generic_affine:
        gb_d = nc.declare_dram_parameter("gbcon", [P, 2 * WIDTH], f32, isOutput=False)
    out_d = nc.declare_dram_parameter("out", [NODES_PER_CORE, WIDTH], f32, isOutput=True)

    with tile.TileContext(nc) as tc:
        with ExitStack() as ctx:
            const = ctx.enter_context(tc.tile_pool(name="const", bufs=1))
            gpool = ctx.enter_context(tc.tile_pool(name="g", bufs=6))
            spool = ctx.enter_context(tc.tile_pool(name="s", bufs=4))
            apool = ctx.enter_context(tc.tile_pool(name="aggT", bufs=3))
            ypool = ctx.enter_context(tc.tile_pool(name="y", bufs=2))
            sqpool = ctx.enter_context(tc.tile_pool(name="sq", bufs=2))
            stat = ctx.enter_context(tc.tile_pool(name="stat", bufs=6))
            ppool = ctx.enter_context(tc.tile_pool(name="psA", bufs=3, space="PSUM"))
            opsum = ctx.enter_context(tc.tile_pool(name="psO", bufs=2, space="PSUM"))

            # idx tables first so the gathers can start ASAP; xself (3.2 MB)
            # last — only the first diag matmul waits on it.
            idxe_sb = const.tile([P, 8 * sTL], i16)
            nc.sync.dma_start(idxe_sb[:], idxe_d[:, :])
            idxo_sb = const.tile([P, 8 * sTH], i16)
            nc.sync.dma_start(idxo_sb[:], idxo_d[:, :])
            fcon_sb = const.tile([P, N_BLOCKS], f32)
            nc.sync.dma_start(fcon_sb[:], fcon_d[:, :])
            wcon_sb = const.tile([P, BW], bf16)
            nc.sync.dma_start(wcon_sb[:], wcon_d[:, :])
            vrow_sb = const.tile([1, VW], bf16)
            nc.sync.dma_start(vrow_sb[:], vrow_d[:, :])
            xself_sb = const.tile([P, N_BLOCKS * W2], bf16)
            nc.sync.dma_start(xself_sb[:], xself_d[:, :])
            if generic_affine:
                gb_sb = const.tile([P, 2 * WIDTH], f32)
                nc.sync.dma_start(gb_sb[:], gb_d[:, :])

            wt_sb = wcon_sb[:, : 2 * NW]
            ident_sb = wcon_sb[:, 2 * NW : 2 * NW + P]

            qn = 0
            for blocks, ne, no in chunks:
                e0 = int(EOFF[blocks[0]])
                o0 = int(OOFF[blocks[0]])
                t0 = int(TOFF[blocks[0]])
                nt = sum(TL[b] + TH[b] for b in blocks)
                ge = go = None
                if ne:
                    ge = gpool.tile([P, ne, WIDTH], bf16, tag="ge")
                    nc.gpsimd.dma_gather(
                        ge[:],
                        xe_d[:, :],
                        idxe_sb[:, 8 * e0 : 8 * (e0 + ne)],
                        ne * P,
                        ne * P,
                        WIDTH,
                        queue_num=qn % 4,
                    )
                    qn += 1
                if no:
                    go = gpool.tile([P, no, WIDTH], bf16, tag="go")
                    nc.gpsimd.dma_gather(
                        go[:],
                        xo_d[:, :],
                        idxo_sb[:, 8 * o0 : 8 * (o0 + no)],
                        no * P,
                        no * P,
                        WIDTH,
                        queue_num=qn % 4,
                    )
                    qn += 1
                sc = spool.tile([P, nt * P], fp8, tag="sc")
                nc.sync.dma_start(sc[:], scon_d[:, t0 * P : (t0 + nt) * P])
                for b in blocks:
                    tl = int(TOFF[b]) - t0
                    seq = [(ge, int(EOFF[b]) - e0 + t) for t in range(TL[b])] + [
                        (go, int(OOFF[b]) - o0 + t) for t in range(TH[b])
                    ]
                    ntb = len(seq)
                    # NOTE: matmul start=True resets the whole PSUM bank, so
                    # the two ch-half accumulation groups need separate tiles
                    # (separate banks), not column windows of one tile.
                    ps0 = ppool.tile([P, P], f32, tag="ps0")
                    ps1 = ppool.tile([P, P], f32, tag="ps1")
                    # self-loop contribution: aggT += (dinv*x_self)^T via identity
                    nc.tensor.matmul(
                        out=ps0[:],
                        lhsT=xself_sb[:, b * W2 : b * W2 + P],
                        rhs=ident_sb,
                        start=True,
                        stop=False,
                    )
                    nc.tensor.matmul(
                        out=ps1[:],
                        lhsT=xself_sb[:, b * W2 + P : (b + 1) * W2],
                        rhs=ident_sb,
                        start=True,
                        stop=False,
                    )
                    for k, (gt, col) in enumerate(seq):
                        s_ap = sc[:, (tl + k) * P : (tl + k + 1) * P]
                        last = k == ntb - 1
                        nc.tensor.matmul(
                            out=ps0[:],
                            lhsT=gt[:, col, 0:P],
                            rhs=s_ap,
                            start=False,
                            stop=last,
                        )
                        nc.tensor.matmul(
                            out=ps1[:],
                            lhsT=gt[:, col, P:WIDTH],
                            rhs=s_ap,
                            start=False,
                            stop=last,
                        )
                    # aggT [2x 128ch, 128 dst] -> SBUF (cast) for the W-matmul
                    a = apool.tile([P, 2 * P], bf16, tag="a")
                    nc.vector.tensor_scalar_mul(a[:, 0:P], ps0[:], 1.0)
                    nc.scalar.copy(a[:, P : 2 * P], ps1[:])
                    po = opsum.tile([P, NW], f32, tag="po")
                    nc.tensor.matmul(
                        out=po[:],
                        lhsT=a[:, 0:P],
                        rhs=wt_sb[:, 0:NW],
                        start=True,
                        stop=False,
                    )
                    nc.tensor.matmul(
                        out=po[:],
                        lhsT=a[:, P : 2 * P],
                        rhs=wt_sb[:, NW : 2 * NW],
                        start=False,
                        stop=False,
                    )
                    # bias row scaled by sqrt(deg) so y = dinv_dst * po exactly
                    nc.tensor.matmul(
                        out=po[:],
                        lhsT=vrow_sb[:, NW + b * P : NW + (b + 1) * P],
                        rhs=vrow_sb[:, 0:NW],
                        start=False,
                        stop=True,
                    )
                    # ---- epilogue: LN + ReLU on y = dinv*po, all per-partition ----
                    sq = sqpool.tile([P, WIDTH], f32, tag="sq")
                    ssq = stat.tile([P, 1], f32, tag="ssq")
                    nc.scalar.activation(
                        out=sq[:], in_=po[:, :WIDTH], func=Act.Square, accum_out=ssq[:]
                    )
                    m2 = stat.tile([P, 1], f32, tag="m2")
                    nc.scalar.activation(
                        out=m2[:],
                        in_=po[:, WIDTH : WIDTH + 1],
                        func=Act.Square,
                        scale=1.0 / WIDTH,
                    )
                    rv = stat.tile([P, 1], f32, tag="rv")
                    nc.vector.tensor_scalar(
                        out=rv[:],
                        in0=ssq[:],
                        scalar1=1.0 / WIDTH,
                        scalar2=m2[:, :1],
                        op0=Alu.mult,
                        op1=Alu.subtract,
                    )
                    sd = stat.tile([P, 1], f32, tag="sd")
                    nc.scalar.activation(
                        out=sd[:], in_=rv[:], func=Act.Sqrt, bias=fcon_sb[:, b : b + 1]
                    )
                    rstd = stat.tile([P, 1], f32, tag="rstd")
                    nc.vector.reciprocal(rstd[:], sd[:])
                    nb = stat.tile([P, 1], f32, tag="nb")
                    nc.vector.tensor_scalar(
                        out=nb[:],
                        in0=po[:, WIDTH : WIDTH + 1],
                        scalar1=-1.0 / WIDTH,
                        scalar2=rstd[:, :1],
                        op0=Alu.mult,
                        op1=Alu.mult,
                    )
                    yo = ypool.tile([P, WIDTH], f32, tag="yo")
                    if not generic_affine:
                        nc.scalar.activation(
                            out=yo[:],
                            in_=po[:, :WIDTH],
                            func=Act.Relu,
                            scale=rstd[:, :1],
                            bias=nb[:, :1],
                        )
                    else:
                        t1 = ypool.tile([P, WIDTH], f32, tag="t1")
                        nc.scalar.activation(
                            out=t1[:],
                            in_=po[:, :WIDTH],
                            func=Act.Identity,
                            scale=rstd[:, :1],
                            bias=nb[:, :1],
                        )
                        t2 = ypool.tile([P, WIDTH], f32, tag="t2")
                        nc.vector.tensor_tensor(
                            out=t2[:], in0=t1[:], in1=gb_sb[:, :WIDTH], op=Alu.mult
                        )
                        t3 = ypool.tile([P, WIDTH], f32, tag="t3")
                        nc.vector.tensor_tensor(
                            out=t3[:], in0=t2[:], in1=gb_sb[:, WIDTH:], op=Alu.add
                        )
                        nc.scalar.activation(out=yo[:], in_=t3[:], func=Act.Relu)
                    rows = min(P, NODES_PER_CORE - b * P)
                    nc.sync.dma_start(out_d[b * P : b * P + rows, :], yo[:rows, :])
    return nc


def _pack_inputs(TL, TH, deg, idxe, idxo, scon, x, W, bias, gamma, beta, generic_affine):
    bfnp = ml_dtypes.bfloat16

    dinv = (1.0 / np.sqrt(deg)).astype(np.float64)
    xs = (np.asarray(x, np.float64) * dinv[:, None]).astype(bfnp)  # dinv-prescaled
    xe = np.ascontiguousarray(xs[0::2])
    xo = np.ascontiguousarray(xs[1::2])

    # xself: per core, [128, 49*256]: xself[p, b*256+ch] = xs[c*6250+b*128+p, ch]
    xself_all = np.zeros((N_CORES, N_BLOCKS, P, WIDTH), bfnp)
    for c in range(N_CORES):
        sl = xs[c * NODES_PER_CORE : (c + 1) * NODES_PER_CORE]
        flat = np.zeros((N_BLOCKS * P, WIDTH), bfnp)
        flat[: NODES_PER_CORE] = sl
        xself_all[c] = flat.reshape(N_BLOCKS, P, WIDTH)
    xself_all = np.ascontiguousarray(
        xself_all.transpose(0, 2, 1, 3).reshape(N_CORES, P, N_BLOCKS * WIDTH)
    )

    # fcon: eps*deg per (partition, block); pads -> deg 1
    degp = np.ones((N_CORES, N_BLOCKS * P), np.float64)
    for c in range(N_CORES):
        degp[c, :NODES_PER_CORE] = deg[c * NODES_PER_CORE : (c + 1) * NODES_PER_CORE]
    epsdeg = (LN_EPS * degp).astype(np.float32).reshape(N_CORES, N_BLOCKS, P)
    epsdeg = np.ascontiguousarray(epsdeg.transpose(0, 2, 1))  # [8, 128, 49]

    WT32 = np.asarray(W, np.float64).T  # [in, out]
    rs = WT32.sum(axis=1, keepdims=True)
    WTe = np.concatenate([WT32, rs], axis=1).astype(bfnp)  # [256, 257]
    wt = np.concatenate([WTe[:P], WTe[P:]], axis=1)  # [128, 514]
    ident = np.eye(P, dtype=bfnp)
    wcon = np.ascontiguousarray(np.concatenate([wt, ident], axis=1))

    b64 = np.asarray(bias, np.float64)
    brow = np.concatenate([b64, [b64.sum()]])  # [257]
    sdeg = np.sqrt(degp)  # [8, N_BLOCKS*P]
    vrow_all = np.concatenate(
        [np.tile(brow[None, :], (N_CORES, 1)), sdeg], axis=1
    ).astype(bfnp)  # [8, 257 + 6272]

    in_maps = []
    for c in range(N_CORES):
        m = {
            "xe": xe,
            "xo": xo,
            "idxe": np.ascontiguousarray(idxe[c]),
            "idxo": np.ascontiguousarray(idxo[c]),
            "scon": np.ascontiguousarray(scon[c]),
            "xself": xself_all[c],
            "fcon": epsdeg[c],
            "wcon": wcon,
            "vrow": vrow_all[c : c + 1],
        }
        if generic_affine:
            gb = np.concatenate(
                [
                    np.tile(np.asarray(gamma, np.float32)[None, :], (P, 1)),
                    np.tile(np.asarray(beta, np.float32)[None, :], (P, 1)),
                ],
                axis=1,
            )
            m["gbcon"] = np.ascontiguousarray(gb)
        in_maps.append(m)
    return in_maps


_PROGRAM_CACHE = {}


def kernel(x, edge_index, W, b, gamma, beta, _run_kwargs=None):
    from concourse.bass_utils import run_bass_kernel_spmd

    x = np.asarray(x)
    W = np.asarray(W)
    bias = np.asarray(b)
    gamma = np.asarray(gamma)
    beta = np.asarray(beta)

    TL, TH, deg, idxe, idxo, scon = _preprocess(edge_index)
    generic_affine = not (np.all(gamma == 1.0) and np.all(beta == 0.0))

    key = (tuple(TL), tuple(TH), generic_affine)
    if key not in _PROGRAM_CACHE:
        nc = _build_program(TL, TH, generic_affine)
        nc.finalize()
        _PROGRAM_CACHE[key] = nc
    nc = _PROGRAM_CACHE[key]

    in_maps = _pack_inputs(
        TL, TH, deg, idxe, idxo, scon, x, W, bias, gamma, beta, generic_affine
    )

    kwargs = dict(_run_kwargs or {})
    kwargs.pop("_result", None)
    rr = run_bass_kernel_spmd(nc, in_maps, list(range(N_CORES)), **kwargs)
    out = np.concatenate([rr.results[c]["out"] for c in range(N_CORES)], axis=0)
    if _run_kwargs is not None:
        _run_kwargs["_result"] = rr
    return np.ascontiguousarray(out.astype(np.float32))


# revision 8
# speedup vs baseline: 1.7248x; 1.2710x over previous
"""GCN block (GCNConv + LayerNorm + ReLU) on 8 Trainium2 NeuronCores.

v3: like v2 (precomputed fp8 one-hot S stream, dinv-prescaled gather
tables, identity-rhs self-loops, fused bias/LN/ReLU epilogue) but the
message streams are PACKED: within a segment of SEG_BLOCKS dst blocks,
each core's messages are laid out back-to-back with NO per-block tile
padding (padding only at segment ends, to the max core's tile count).
Block -> tile mapping becomes data-dependent, so each block's PSUM
accumulation covers the UNION across cores of the tiles its messages can
land in (a static interval, from the per-core cumulative counts); the
per-core S tiles zero out the slots that belong to other blocks.
"""

import math
import sys

sys.path.insert(0, "/opt/trn_rl_repo")

import numpy as np
import ml_dtypes

N_NODES = 50000
WIDTH = 256
N_CORES = 8
NODES_PER_CORE = N_NODES // N_CORES  # 6250
P = 128
N_BLOCKS = math.ceil(NODES_PER_CORE / P)  # 49 (last block has 106 rows)
LN_EPS = 1e-5
HALF = N_NODES // 2  # rows per gather table
SEG_BLOCKS = 2  # dst blocks packed per gather segment (keeps tiles <= 8)


def _preprocess(edge_index):
    """Pack non-self-loop messages per (core, segment, parity) contiguously.

    Returns (meta, deg, idxe, idxo, scon) where meta carries the static
    structure shared by all cores:
      meta = (segtiles_e, segtiles_o,   # [n_segs] tiles per segment/parity
              lo_e, hi_e, lo_o, hi_o,   # [N_BLOCKS] participation intervals
                                        # (tile idx relative to segment start)
              soff)                     # [N_BLOCKS] S-stream tile offset
    """
    src = np.asarray(edge_index[0]).astype(np.int64)
    dst = np.asarray(edge_index[1]).astype(np.int64)

    deg = (np.bincount(dst, minlength=N_NODES) + 1).astype(np.float64)  # + self loop

    core = dst // NODES_PER_CORE
    r = dst % NODES_PER_CORE
    blk = np.minimum(r // P, N_BLOCKS - 1)
    dcol = r - blk * P
    tab = src & 1
    gbin = (core * N_BLOCKS + blk) * 2 + tab

    order = np.argsort(gbin, kind="stable")
    src, dcol, gbin = src[order], dcol[order], gbin[order]
    c = gbin // (N_BLOCKS * 2)
    b = (gbin // 2) % N_BLOCKS
    t = gbin & 1

    cnt = np.bincount(gbin, minlength=N_CORES * N_BLOCKS * 2).reshape(
        N_CORES, N_BLOCKS, 2
    )
    n_segs = math.ceil(N_BLOCKS / SEG_BLOCKS)
    seg_of = np.arange(N_BLOCKS) // SEG_BLOCKS

    # per-core cumulative counts within each segment -> slot of each message
    # start[c, b, t] = within-segment start slot of block b's bucket
    start = np.zeros((N_CORES, N_BLOCKS, 2), np.int64)
    for s in range(n_segs):
        bs = np.where(seg_of == s)[0]
        cum = np.cumsum(cnt[:, bs, :], axis=1)
        start[:, bs[1:], :] = cum[:, :-1, :]
    end = start + cnt  # within-segment end slot

    segtiles = np.zeros((n_segs, 2), np.int64)
    for s in range(n_segs):
        bs = np.where(seg_of == s)[0]
        tot = cnt[:, bs, :].sum(axis=1)  # [8, 2]
        segtiles[s] = np.ceil(tot.max(axis=0) / P).astype(np.int64)
    segtiles_e = segtiles[:, 0]
    segtiles_o = segtiles[:, 1]

    # participation intervals (tiles relative to segment start)
    lo = np.zeros((N_BLOCKS, 2), np.int64)
    hi = np.zeros((N_BLOCKS, 2), np.int64)
    for bb in range(N_BLOCKS):
        s = seg_of[bb]
        for tt in range(2):
            lo[bb, tt] = start[:, bb, tt].min() // P
            hi[bb, tt] = min(
                int(np.ceil(end[:, bb, tt].max() / P)), int(segtiles[s, tt])
            )
            hi[bb, tt] = max(hi[bb, tt], lo[bb, tt])  # empty-bucket guard
    we = (hi - lo)[:, 0]
    wo = (hi - lo)[:, 1]
    soff = np.concatenate([[0], np.cumsum(we + wo)])  # S tile offset per block
    s_tiles_tot = int(soff[-1])

    # gather-stream tile offsets per segment
    egoff = np.concatenate([[0], np.cumsum(segtiles_e)])
    ogoff = np.concatenate([[0], np.cumsum(segtiles_o)])
    sTL, sTH = int(egoff[-1]), int(ogoff[-1])

    # per-message placement
    starts_flat = start[c, b, t]  # within-seg start slot of this bucket
    jj = np.zeros(len(gbin), np.int64)
    bstarts = np.concatenate([[0], np.cumsum(cnt.ravel())])[:-1]
    jj = np.arange(len(gbin)) - bstarts[gbin]  # index within bucket
    slot = starts_flat + jj  # within-segment slot
    tile_in_seg = slot // P
    p = slot % P

    idxe_flat = np.zeros((N_CORES, sTL * P), np.int16)
    idxo_flat = np.zeros((N_CORES, sTH * P), np.int16)
    scon = np.zeros((N_CORES, P, s_tiles_tot * P), ml_dtypes.float8_e4m3)

    idx16 = (src >> 1).astype(np.int16)
    seg_m = seg_of[b]
    ev = t == 0
    Je = (egoff[seg_m] + tile_in_seg) * P + p
    Jo = (ogoff[seg_m] + tile_in_seg) * P + p
    idxe_flat[c[ev], Je[ev]] = idx16[ev]
    idxo_flat[c[~ev], Jo[~ev]] = idx16[~ev]

    # S stream: per block, even participation tiles then odd
    spos = np.where(
        ev,
        soff[b] + (tile_in_seg - lo[b, 0]),
        soff[b] + we[b] + (tile_in_seg - lo[b, 1]),
    )
    scon[c, p, spos * P + dcol] = 1.0

    def wrap(flat, ntiles):
        if ntiles == 0:
            return np.zeros((N_CORES, P, 0), np.int16)
        a = flat.reshape(N_CORES, ntiles * 8, 16).transpose(0, 2, 1)
        return np.ascontiguousarray(np.tile(a, (1, 8, 1)))

    meta = (
        tuple(int(v) for v in segtiles_e),
        tuple(int(v) for v in segtiles_o),
        tuple(int(v) for v in lo[:, 0]),
        tuple(int(v) for v in hi[:, 0]),
        tuple(int(v) for v in lo[:, 1]),
        tuple(int(v) for v in hi[:, 1]),
        tuple(int(v) for v in soff),
    )
    return meta, deg, wrap(idxe_flat, sTL), wrap(idxo_flat, sTH), scon


def _build_program(meta, generic_affine):
    import concourse.bass as bass
    import concourse.tile as tile
    from concourse import bacc as bacc_mod
    from concourse import mybir
    from contextlib import ExitStack

    f32 = mybir.dt.float32
    bf16 = mybir.dt.bfloat16
    fp8 = mybir.dt.float8e4
    i16 = mybir.dt.int16
    Alu = mybir.AluOpType
    Act = mybir.ActivationFunctionType

    segtiles_e, segtiles_o, lo_e, hi_e, lo_o, hi_o, soff = meta
    n_segs = len(segtiles_e)
    seg_of = [bb // SEG_BLOCKS for bb in range(N_BLOCKS)]
    egoff = np.concatenate([[0], np.cumsum(segtiles_e)]).astype(int)
    ogoff = np.concatenate([[0], np.cumsum(segtiles_o)]).astype(int)
    sTL, sTH = int(egoff[-1]), int(ogoff[-1])
    s_tiles_tot = int(soff[-1])

    W2 = WIDTH
    BW = 2 * (WIDTH + 1) + P
    VW = (WIDTH + 1) + N_BLOCKS * P
    NW = WIDTH + 1  # 257

    nc = bacc_mod.Bacc(None, target_bir_lowering=False, debug=False, num_swdge_queues=4)
    xe_d = nc.declare_dram_parameter("xe", [HALF, WIDTH], bf16, isOutput=False)
    xo_d = nc.declare_dram_parameter("xo", [HALF, WIDTH], bf16, isOutput=False)
    idxe_d = nc.declare_dram_parameter("idxe", [P, 8 * sTL], i16, isOutput=False)
    idxo_d = nc.declare_dram_parameter("idxo", [P, 8 * sTH], i16, isOutput=False)
    scon_d = nc.declare_dram_parameter("scon", [P, s_tiles_tot * P], fp8, isOutput=False)
    xself_d = nc.declare_dram_parameter("xself", [P, N_BLOCKS * W2], bf16, isOutput=False)
    fcon_d = nc.declare_dram_parameter("fcon", [P, N_BLOCKS], f32, isOutput=False)
    wcon_d = nc.declare_dram_parameter("wcon", [P, BW], bf16, isOutput=False)
    vrow_d = nc.declare_dram_parameter("vrow", [1, VW], bf16, isOutput=False)
    if generic_affine:
        gb_d = nc.declare_dram_parameter("gbcon", [P, 2 * WIDTH], f32, isOutput=False)
    out_d = nc.declare_dram_parameter("out", [NODES_PER_CORE, WIDTH], f32, isOutput=True)

    with tile.TileContext(nc) as tc:
        with ExitStack() as ctx:
            const = ctx.enter_context(tc.tile_pool(name="const", bufs=1))
            gpool = ctx.enter_context(tc.tile_pool(name="g", bufs=6))
            spool = ctx.enter_context(tc.tile_pool(name="s", bufs=4))
            apool = ctx.enter_context(tc.tile_pool(name="aggT", bufs=3))
            ypool = ctx.enter_context(tc.tile_pool(name="y", bufs=2))
            sqpool = ctx.enter_context(tc.tile_pool(name="sq", bufs=2))
            stat = ctx.enter_context(tc.tile_pool(name="stat", bufs=6))
            ppool = ctx.enter_context(tc.tile_pool(name="psA", bufs=3, space="PSUM"))
            opsum = ctx.enter_context(tc.tile_pool(name="psO", bufs=2, space="PSUM"))

            idxe_sb = const.tile([P, 8 * sTL], i16)
            nc.sync.dma_start(idxe_sb[:], idxe_d[:, :])
            idxo_sb = const.tile([P, 8 * sTH], i16)
            nc.sync.dma_start(idxo_sb[:], idxo_d[:, :])
            fcon_sb = const.tile([P, N_BLOCKS], f32)
            nc.sync.dma_start(fcon_sb[:], fcon_d[:, :])
            wcon_sb = const.tile([P, BW], bf16)
            nc.sync.dma_start(wcon_sb[:], wcon_d[:, :])
            vrow_sb = const.tile([1, VW], bf16)
            nc.sync.dma_start(vrow_sb[:], vrow_d[:, :])
            xself_sb = const.tile([P, N_BLOCKS * W2], bf16)
            nc.sync.dma_start(xself_sb[:], xself_d[:, :])
            if generic_affine:
                gb_sb = const.tile([P, 2 * WIDTH], f32)
                nc.sync.dma_start(gb_sb[:], gb_d[:, :])

            wt_sb = wcon_sb[:, : 2 * NW]
            ident_sb = wcon_sb[:, 2 * NW : 2 * NW + P]

            qn = 0
            for s in range(n_segs):
                ne, no = segtiles_e[s], segtiles_o[s]
                blocks = [bb for bb in range(N_BLOCKS) if seg_of[bb] == s]
                ge = go = None
                if ne:
                    ge = gpool.tile([P, ne, WIDTH], bf16, tag="ge")
                    nc.gpsimd.dma_gather(
                        ge[:],
                        xe_d[:, :],
                        idxe_sb[:, 8 * int(egoff[s]) : 8 * int(egoff[s] + ne)],
                        ne * P,
                        ne * P,
                        WIDTH,
                        queue_num=qn % 4,
                    )
                    qn += 1
                if no:
                    go = gpool.tile([P, no, WIDTH], bf16, tag="go")
                    nc.gpsimd.dma_gather(
                        go[:],
                        xo_d[:, :],
                        idxo_sb[:, 8 * int(ogoff[s]) : 8 * int(ogoff[s] + no)],
                        no * P,
                        no * P,
                        WIDTH,
                        queue_num=qn % 4,
                    )
                    qn += 1
                st0 = int(soff[blocks[0]])
                nst = int(soff[blocks[-1] + 1]) - st0
                sc = spool.tile([P, nst * P], fp8, tag="sc")
                nc.sync.dma_start(sc[:], scon_d[:, st0 * P : (st0 + nst) * P])
                for bb in blocks:
                    seq = [(ge, tt) for tt in range(lo_e[bb], hi_e[bb])] + [
                        (go, tt) for tt in range(lo_o[bb], hi_o[bb])
                    ]
                    ntb = len(seq)
                    ps0 = ppool.tile([P, P], f32, tag="ps0")
                    ps1 = ppool.tile([P, P], f32, tag="ps1")
                    nc.tensor.matmul(
                        out=ps0[:],
                        lhsT=xself_sb[:, bb * W2 : bb * W2 + P],
                        rhs=ident_sb,
                        start=True,
                        stop=(ntb == 0),
                    )
                    nc.tensor.matmul(
                        out=ps1[:],
                        lhsT=xself_sb[:, bb * W2 + P : (bb + 1) * W2],
                        rhs=ident_sb,
                        start=True,
                        stop=(ntb == 0),
                    )
                    for k, (gt, col) in enumerate(seq):
                        s_ap = sc[:, (int(soff[bb]) - st0 + k) * P : (int(soff[bb]) - st0 + k + 1) * P]
                        last = k == ntb - 1
                        nc.tensor.matmul(
                            out=ps0[:],
                            lhsT=gt[:, col, 0:P],
                            rhs=s_ap,
                            start=False,
                            stop=last,
                        )
                        nc.tensor.matmul(
                            out=ps1[:],
                            lhsT=gt[:, col, P:WIDTH],
                            rhs=s_ap,
                            start=False,
                            stop=last,
                        )
                    a = apool.tile([P, 2 * P], bf16, tag="a")
                    nc.vector.tensor_scalar_mul(a[:, 0:P], ps0[:], 1.0)
                    nc.scalar.copy(a[:, P : 2 * P], ps1[:])
                    po = opsum.tile([P, NW], f32, tag="po")
                    nc.tensor.matmul(
                        out=po[:],
                        lhsT=a[:, 0:P],
                        rhs=wt_sb[:, 0:NW],
                        start=True,
                        stop=False,
                    )
                    nc.tensor.matmul(
                        out=po[:],
                        lhsT=a[:, P : 2 * P],
                        rhs=wt_sb[:, NW : 2 * NW],
                        start=False,
                        stop=False,
                    )
                    nc.tensor.matmul(
                        out=po[:],
                        lhsT=vrow_sb[:, NW + bb * P : NW + (bb + 1) * P],
                        rhs=vrow_sb[:, 0:NW],
                        start=False,
                        stop=True,
                    )
                    sq = sqpool.tile([P, WIDTH], f32, tag="sq")
                    ssq = stat.tile([P, 1], f32, tag="ssq")
                    nc.scalar.activation(
                        out=sq[:], in_=po[:, :WIDTH], func=Act.Square, accum_out=ssq[:]
                    )
                    m2 = stat.tile([P, 1], f32, tag="m2")
                    nc.scalar.activation(
                        out=m2[:],
                        in_=po[:, WIDTH : WIDTH + 1],
                        func=Act.Square,
                        scale=1.0 / WIDTH,
                    )
                    rv = stat.tile([P, 1], f32, tag="rv")
                    nc.vector.tensor_scalar(
                        out=rv[:],
                        in0=ssq[:],
                        scalar1=1.0 / WIDTH,
                        scalar2=m2[:, :1],
                        op0=Alu.mult,
                        op1=Alu.subtract,
                    )
                    sd = stat.tile([P, 1], f32, tag="sd")
                    nc.scalar.activation(
                        out=sd[:], in_=rv[:], func=Act.Sqrt, bias=fcon_sb[:, bb : bb + 1]
                    )
                    rstd = stat.tile([P, 1], f32, tag="rstd")
                    nc.vector.reciprocal(rstd[:], sd[:])
                    nb = stat.tile([P, 1], f32, tag="nb")
                    nc.vector.tensor_scalar(
                        out=nb[:],
                        in0=po[:, WIDTH : WIDTH + 1],
                        scalar1=-1.0 / WIDTH,
                        scalar2=rstd[:, :1],
                        op0=Alu.mult,
                        op1=Alu.mult,
                    )
                    yo = ypool.tile([P, WIDTH], f32, tag="yo")
                    if not generic_affine:
                        nc.scalar.activation(
                            out=yo[:],
                            in_=po[:, :WIDTH],
                            func=Act.Relu,
                            scale=rstd[:, :1],
                            bias=nb[:, :1],
                        )
                    else:
                        t1 = ypool.tile([P, WIDTH], f32, tag="t1")
                        nc.scalar.activation(
                            out=t1[:],
                            in_=po[:, :WIDTH],
                            func=Act.Identity,
                            scale=rstd[:, :1],
                            bias=nb[:, :1],
                        )
                        t2 = ypool.tile([P, WIDTH], f32, tag="t2")
                        nc.vector.tensor_tensor(
                            out=t2[:], in0=t1[:], in1=gb_sb[:, :WIDTH], op=Alu.mult
                        )
                        t3 = ypool.tile([P, WIDTH], f32, tag="t3")
                        nc.vector.tensor_tensor(
                            out=t3[:], in0=t2[:], in1=gb_sb[:, WIDTH:], op=Alu.add
                        )
                        nc.scalar.activation(out=yo[:], in_=t3[:], func=Act.Relu)
                    rows = min(P, NODES_PER_CORE - bb * P)
                    nc.sync.dma_start(out_d[bb * P : bb * P + rows, :], yo[:rows, :])
    return nc


def _pack_inputs(meta, deg, idxe, idxo, scon, x, W, bias, gamma, beta, generic_affine):
    bfnp = ml_dtypes.bfloat16

    dinv = (1.0 / np.sqrt(deg)).astype(np.float64)
    xs = (np.asarray(x, np.float64) * dinv[:, None]).astype(bfnp)
    xe = np.ascontiguousarray(xs[0::2])
    xo = np.ascontiguousarray(xs[1::2])

    xself_all = np.zeros((N_CORES, N_BLOCKS, P, WIDTH), bfnp)
    for c in range(N_CORES):
        sl = xs[c * NODES_PER_CORE : (c + 1) * NODES_PER_CORE]
        flat = np.zeros((N_BLOCKS * P, WIDTH), bfnp)
        flat[: NODES_PER_CORE] = sl
        xself_all[c] = flat.reshape(N_BLOCKS, P, WIDTH)
    xself_all = np.ascontiguousarray(
        xself_all.transpose(0, 2, 1, 3).reshape(N_CORES, P, N_BLOCKS * WIDTH)
    )

    degp = np.ones((N_CORES, N_BLOCKS * P), np.float64)
    for c in range(N_CORES):
        degp[c, :NODES_PER_CORE] = deg[c * NODES_PER_CORE : (c + 1) * NODES_PER_CORE]
    epsdeg = (LN_EPS * degp).astype(np.float32).reshape(N_CORES, N_BLOCKS, P)
    epsdeg = np.ascontiguousarray(epsdeg.transpose(0, 2, 1))

    WT32 = np.asarray(W, np.float64).T
    rs = WT32.sum(axis=1, keepdims=True)
    WTe = np.concatenate([WT32, rs], axis=1).astype(bfnp)
    wt = np.concatenate([WTe[:P], WTe[P:]], axis=1)
    ident = np.eye(P, dtype=bfnp)
    wcon = np.ascontiguousarray(np.concatenate([wt, ident], axis=1))

    b64 = np.asarray(bias, np.float64)
    brow = np.concatenate([b64, [b64.sum()]])
    sdeg = np.sqrt(degp)
    vrow_all = np.concatenate(
        [np.tile(brow[None, :], (N_CORES, 1)), sdeg], axis=1
    ).astype(bfnp)

    in_maps = []
    for c in range(N_CORES):
        m = {
            "xe": xe,
            "xo": xo,
            "idxe": np.ascontiguousarray(idxe[c]),
            "idxo": np.ascontiguousarray(idxo[c]),
            "scon": np.ascontiguousarray(scon[c]),
            "xself": xself_all[c],
            "fcon": epsdeg[c],
            "wcon": wcon,
            "vrow": vrow_all[c : c + 1],
        }
        if generic_affine:
            gb = np.concatenate(
                [
                    np.tile(np.asarray(gamma, np.float32)[None, :], (P, 1)),
                    np.tile(np.asarray(beta, np.float32)[None, :], (P, 1)),
                ],
                axis=1,
            )
            m["gbcon"] = np.ascontiguousarray(gb)
        in_maps.append(m)
    return in_maps


_PROGRAM_CACHE = {}


def kernel(x, edge_index, W, b, gamma, beta, _run_kwargs=None):
    from concourse.bass_utils import run_bass_kernel_spmd

    x = np.asarray(x)
    W = np.asarray(W)
    bias = np.asarray(b)
    gamma = np.asarray(gamma)
    beta = np.asarray(beta)

    meta, deg, idxe, idxo, scon = _preprocess(edge_index)
    generic_affine = not (np.all(gamma == 1.0) and np.all(beta == 0.0))

    key = (meta, generic_affine)
    if key not in _PROGRAM_CACHE:
        nc = _build_program(meta, generic_affine)
        nc.finalize()
        _PROGRAM_CACHE[key] = nc
    nc = _PROGRAM_CACHE[key]

    in_maps = _pack_inputs(
        meta, deg, idxe, idxo, scon, x, W, bias, gamma, beta, generic_affine
    )

    kwargs = dict(_run_kwargs or {})
    kwargs.pop("_result", None)
    rr = run_bass_kernel_spmd(nc, in_maps, list(range(N_CORES)), **kwargs)
    out = np.concatenate([rr.results[c]["out"] for c in range(N_CORES)], axis=0)
    if _run_kwargs is not None:
        _run_kwargs["_result"] = rr
    return np.ascontiguousarray(out.astype(np.float32))


# revision 10
# speedup vs baseline: 1.7426x; 1.0103x over previous
"""GCN block (GCNConv + LayerNorm + ReLU) on 8 Trainium2 NeuronCores.

v3: like v2 (precomputed fp8 one-hot S stream, dinv-prescaled gather
tables, identity-rhs self-loops, fused bias/LN/ReLU epilogue) but the
message streams are PACKED: within a segment of SEG_BLOCKS dst blocks,
each core's messages are laid out back-to-back with NO per-block tile
padding (padding only at segment ends, to the max core's tile count).
Block -> tile mapping becomes data-dependent, so each block's PSUM
accumulation covers the UNION across cores of the tiles its messages can
land in (a static interval, from the per-core cumulative counts); the
per-core S tiles zero out the slots that belong to other blocks.
"""

import math
import sys

sys.path.insert(0, "/opt/trn_rl_repo")

import numpy as np
import ml_dtypes

N_NODES = 50000
WIDTH = 256
N_CORES = 8
NODES_PER_CORE = N_NODES // N_CORES  # 6250
P = 128
N_BLOCKS = math.ceil(NODES_PER_CORE / P)  # 49 (last block has 106 rows)
LN_EPS = 1e-5
HALF = N_NODES // 2  # rows per gather table
SEG_BLOCKS = 2  # dst blocks packed per gather segment (keeps tiles <= 8)


def _preprocess(edge_index):
    """Pack non-self-loop messages per (core, segment, parity) contiguously.

    Returns (meta, deg, idxe, idxo, scon) where meta carries the static
    structure shared by all cores:
      meta = (segtiles_e, segtiles_o,   # [n_segs] tiles per segment/parity
              lo_e, hi_e, lo_o, hi_o,   # [N_BLOCKS] participation intervals
                                        # (tile idx relative to segment start)
              soff)                     # [N_BLOCKS] S-stream tile offset
    """
    src = np.asarray(edge_index[0]).astype(np.int64)
    dst = np.asarray(edge_index[1]).astype(np.int64)

    deg = (np.bincount(dst, minlength=N_NODES) + 1).astype(np.float64)  # + self loop

    core = dst // NODES_PER_CORE
    r = dst % NODES_PER_CORE
    blk = np.minimum(r // P, N_BLOCKS - 1)
    dcol = r - blk * P
    tab = src & 1
    gbin = (core * N_BLOCKS + blk) * 2 + tab

    order = np.argsort(gbin, kind="stable")
    src, dcol, gbin = src[order], dcol[order], gbin[order]
    c = gbin // (N_BLOCKS * 2)
    b = (gbin // 2) % N_BLOCKS
    t = gbin & 1

    cnt = np.bincount(gbin, minlength=N_CORES * N_BLOCKS * 2).reshape(
        N_CORES, N_BLOCKS, 2
    )
    n_segs = math.ceil(N_BLOCKS / SEG_BLOCKS)
    seg_of = np.arange(N_BLOCKS) // SEG_BLOCKS

    # per-core cumulative counts within each segment -> slot of each message
    # start[c, b, t] = within-segment start slot of block b's bucket
    start = np.zeros((N_CORES, N_BLOCKS, 2), np.int64)
    for s in range(n_segs):
        bs = np.where(seg_of == s)[0]
        cum = np.cumsum(cnt[:, bs, :], axis=1)
        start[:, bs[1:], :] = cum[:, :-1, :]
    end = start + cnt  # within-segment end slot

    segtiles = np.zeros((n_segs, 2), np.int64)
    for s in range(n_segs):
        bs = np.where(seg_of == s)[0]
        tot = cnt[:, bs, :].sum(axis=1)  # [8, 2]
        segtiles[s] = np.ceil(tot.max(axis=0) / P).astype(np.int64)
    segtiles_e = segtiles[:, 0]
    segtiles_o = segtiles[:, 1]

    # participation intervals (tiles relative to segment start)
    lo = np.zeros((N_BLOCKS, 2), np.int64)
    hi = np.zeros((N_BLOCKS, 2), np.int64)
    for bb in range(N_BLOCKS):
        s = seg_of[bb]
        for tt in range(2):
            lo[bb, tt] = start[:, bb, tt].min() // P
            hi[bb, tt] = min(
                int(np.ceil(end[:, bb, tt].max() / P)), int(segtiles[s, tt])
            )
            hi[bb, tt] = max(hi[bb, tt], lo[bb, tt])  # empty-bucket guard
    we = (hi - lo)[:, 0]
    wo = (hi - lo)[:, 1]
    soff = np.concatenate([[0], np.cumsum(we + wo)])  # S tile offset per block
    s_tiles_tot = int(soff[-1])

    # gather-stream tile offsets per segment
    egoff = np.concatenate([[0], np.cumsum(segtiles_e)])
    ogoff = np.concatenate([[0], np.cumsum(segtiles_o)])
    sTL, sTH = int(egoff[-1]), int(ogoff[-1])

    # per-message placement
    starts_flat = start[c, b, t]  # within-seg start slot of this bucket
    jj = np.zeros(len(gbin), np.int64)
    bstarts = np.concatenate([[0], np.cumsum(cnt.ravel())])[:-1]
    jj = np.arange(len(gbin)) - bstarts[gbin]  # index within bucket
    slot = starts_flat + jj  # within-segment slot
    tile_in_seg = slot // P
    p = slot % P

    idxe_flat = np.zeros((N_CORES, sTL * P), np.int16)
    idxo_flat = np.zeros((N_CORES, sTH * P), np.int16)
    scon = np.zeros((N_CORES, P, s_tiles_tot * P), ml_dtypes.float8_e4m3)

    idx16 = (src >> 1).astype(np.int16)
    seg_m = seg_of[b]
    ev = t == 0
    Je = (egoff[seg_m] + tile_in_seg) * P + p
    Jo = (ogoff[seg_m] + tile_in_seg) * P + p
    idxe_flat[c[ev], Je[ev]] = idx16[ev]
    idxo_flat[c[~ev], Jo[~ev]] = idx16[~ev]

    # S stream: per block, even participation tiles then odd
    spos = np.where(
        ev,
        soff[b] + (tile_in_seg - lo[b, 0]),
        soff[b] + we[b] + (tile_in_seg - lo[b, 1]),
    )
    scon[c, p, spos * P + dcol] = 1.0

    def wrap(flat, ntiles):
        if ntiles == 0:
            return np.zeros((N_CORES, P, 0), np.int16)
        a = flat.reshape(N_CORES, ntiles * 8, 16).transpose(0, 2, 1)
        return np.ascontiguousarray(np.tile(a, (1, 8, 1)))

    meta = (
        tuple(int(v) for v in segtiles_e),
        tuple(int(v) for v in segtiles_o),
        tuple(int(v) for v in lo[:, 0]),
        tuple(int(v) for v in hi[:, 0]),
        tuple(int(v) for v in lo[:, 1]),
        tuple(int(v) for v in hi[:, 1]),
        tuple(int(v) for v in soff),
    )
    return meta, deg, wrap(idxe_flat, sTL), wrap(idxo_flat, sTH), scon


def _build_program(meta, generic_affine):
    import concourse.bass as bass
    import concourse.tile as tile
    from concourse import bacc as bacc_mod
    from concourse import mybir
    from contextlib import ExitStack

    f32 = mybir.dt.float32
    bf16 = mybir.dt.bfloat16
    fp8 = mybir.dt.float8e4
    i16 = mybir.dt.int16
    Alu = mybir.AluOpType
    Act = mybir.ActivationFunctionType

    segtiles_e, segtiles_o, lo_e, hi_e, lo_o, hi_o, soff = meta
    n_segs = len(segtiles_e)
    seg_of = [bb // SEG_BLOCKS for bb in range(N_BLOCKS)]
    egoff = np.concatenate([[0], np.cumsum(segtiles_e)]).astype(int)
    ogoff = np.concatenate([[0], np.cumsum(segtiles_o)]).astype(int)
    sTL, sTH = int(egoff[-1]), int(ogoff[-1])
    s_tiles_tot = int(soff[-1])

    W2 = WIDTH
    BW = 2 * (WIDTH + 1) + P
    VW = (WIDTH + 1) + N_BLOCKS * P
    NW = WIDTH + 1  # 257

    nc = bacc_mod.Bacc(None, target_bir_lowering=False, debug=False, num_swdge_queues=4)
    xe_d = nc.declare_dram_parameter("xe", [HALF, WIDTH], bf16, isOutput=False)
    xo_d = nc.declare_dram_parameter("xo", [HALF, WIDTH], bf16, isOutput=False)
    idxe_d = nc.declare_dram_parameter("idxe", [P, 8 * sTL], i16, isOutput=False)
    idxo_d = nc.declare_dram_parameter("idxo", [P, 8 * sTH], i16, isOutput=False)
    scon_d = nc.declare_dram_parameter("scon", [P, s_tiles_tot * P], fp8, isOutput=False)
    xself_d = nc.declare_dram_parameter("xself", [P, N_BLOCKS * W2], bf16, isOutput=False)
    fcon_d = nc.declare_dram_parameter("fcon", [P, N_BLOCKS], f32, isOutput=False)
    wcon_d = nc.declare_dram_parameter("wcon", [P, BW], bf16, isOutput=False)
    vrow_d = nc.declare_dram_parameter("vrow", [1, VW], bf16, isOutput=False)
    if generic_affine:
        gb_d = nc.declare_dram_parameter("gbcon", [P, 2 * WIDTH], f32, isOutput=False)
    out_d = nc.declare_dram_parameter("out", [NODES_PER_CORE, WIDTH], f32, isOutput=True)

    with tile.TileContext(nc) as tc:
        with ExitStack() as ctx:
            const = ctx.enter_context(tc.tile_pool(name="const", bufs=1))
            gpool = ctx.enter_context(tc.tile_pool(name="g", bufs=6))
            spool = ctx.enter_context(tc.tile_pool(name="s", bufs=4))
            apool = ctx.enter_context(tc.tile_pool(name="aggT", bufs=3))
            ypool = ctx.enter_context(tc.tile_pool(name="y", bufs=2))
            sqpool = ctx.enter_context(tc.tile_pool(name="sq", bufs=2))
            stat = ctx.enter_context(tc.tile_pool(name="stat", bufs=6))
            ppool = ctx.enter_context(tc.tile_pool(name="psA", bufs=3, space="PSUM"))
            opsum = ctx.enter_context(tc.tile_pool(name="psO", bufs=2, space="PSUM"))

            idxe_sb = const.tile([P, 8 * sTL], i16)
            nc.sync.dma_start(idxe_sb[:], idxe_d[:, :])
            idxo_sb = const.tile([P, 8 * sTH], i16)
            nc.sync.dma_start(idxo_sb[:], idxo_d[:, :])
            fcon_sb = const.tile([P, N_BLOCKS], f32)
            nc.sync.dma_start(fcon_sb[:], fcon_d[:, :])
            wcon_sb = const.tile([P, BW], bf16)
            nc.sync.dma_start(wcon_sb[:], wcon_d[:, :])
            vrow_sb = const.tile([1, VW], bf16)
            nc.sync.dma_start(vrow_sb[:], vrow_d[:, :])
            xself_sb = const.tile([P, N_BLOCKS * W2], bf16)
            nc.sync.dma_start(xself_sb[:], xself_d[:, :])
            if generic_affine:
                gb_sb = const.tile([P, 2 * WIDTH], f32)
                nc.sync.dma_start(gb_sb[:], gb_d[:, :])

            wt_sb = wcon_sb[:, : 2 * NW]
            ident_sb = wcon_sb[:, 2 * NW : 2 * NW + P]

            qn = 0
            for s in range(n_segs):
                ne, no = segtiles_e[s], segtiles_o[s]
                blocks = [bb for bb in range(N_BLOCKS) if seg_of[bb] == s]
                ge = go = None
                # each (segment, table) gather is split into two calls so the
                # per-call desc-gen + drain latency halves and more calls are
                # in flight across the 4 SWDGE queues
                if ne:
                    ge = gpool.tile([P, ne, WIDTH], bf16, tag="ge")
                    h = (ne + 1) // 2
                    for c0, c1 in ((0, h), (h, ne)):
                        if c1 == c0:
                            continue
                        nc.gpsimd.dma_gather(
                            ge[:, c0:c1, :],
                            xe_d[:, :],
                            idxe_sb[:, 8 * int(egoff[s] + c0) : 8 * int(egoff[s] + c1)],
                            (c1 - c0) * P,
                            (c1 - c0) * P,
                            WIDTH,
                            queue_num=qn % 4,
                        )
                        qn += 1
                if no:
                    go = gpool.tile([P, no, WIDTH], bf16, tag="go")
                    h = (no + 1) // 2
                    for c0, c1 in ((0, h), (h, no)):
                        if c1 == c0:
                            continue
                        nc.gpsimd.dma_gather(
                            go[:, c0:c1, :],
                            xo_d[:, :],
                            idxo_sb[:, 8 * int(ogoff[s] + c0) : 8 * int(ogoff[s] + c1)],
                            (c1 - c0) * P,
                            (c1 - c0) * P,
                            WIDTH,
                            queue_num=qn % 4,
                        )
                        qn += 1
                st0 = int(soff[blocks[0]])
                nst = int(soff[blocks[-1] + 1]) - st0
                sc = spool.tile([P, nst * P], fp8, tag="sc")
                nc.sync.dma_start(sc[:], scon_d[:, st0 * P : (st0 + nst) * P])
                for bb in blocks:
                    seq = [(ge, tt) for tt in range(lo_e[bb], hi_e[bb])] + [
                        (go, tt) for tt in range(lo_o[bb], hi_o[bb])
                    ]
                    ntb = len(seq)
                    ps0 = ppool.tile([P, P], f32, tag="ps0")
                    ps1 = ppool.tile([P, P], f32, tag="ps1")
                    nc.tensor.matmul(
                        out=ps0[:],
                        lhsT=xself_sb[:, bb * W2 : bb * W2 + P],
                        rhs=ident_sb,
                        start=True,
                        stop=(ntb == 0),
                    )
                    nc.tensor.matmul(
                        out=ps1[:],
                        lhsT=xself_sb[:, bb * W2 + P : (bb + 1) * W2],
                        rhs=ident_sb,
                        start=True,
                        stop=(ntb == 0),
                    )
                    for k, (gt, col) in enumerate(seq):
                        s_ap = sc[:, (int(soff[bb]) - st0 + k) * P : (int(soff[bb]) - st0 + k + 1) * P]
                        last = k == ntb - 1
                        nc.tensor.matmul(
                            out=ps0[:],
                            lhsT=gt[:, col, 0:P],
                            rhs=s_ap,
                            start=False,
                            stop=last,
                        )
                        nc.tensor.matmul(
                            out=ps1[:],
                            lhsT=gt[:, col, P:WIDTH],
                            rhs=s_ap,
                            start=False,
                            stop=last,
                        )
                    a = apool.tile([P, 2 * P], bf16, tag="a")
                    nc.vector.tensor_scalar_mul(a[:, 0:P], ps0[:], 1.0)
                    nc.vector.tensor_scalar_mul(a[:, P : 2 * P], ps1[:], 1.0)
                    po = opsum.tile([P, NW], f32, tag="po")
                    nc.tensor.matmul(
                        out=po[:],
                        lhsT=a[:, 0:P],
                        rhs=wt_sb[:, 0:NW],
                        start=True,
                        stop=False,
                    )
                    nc.tensor.matmul(
                        out=po[:],
                        lhsT=a[:, P : 2 * P],
                        rhs=wt_sb[:, NW : 2 * NW],
                        start=False,
                        stop=False,
                    )
                    nc.tensor.matmul(
                        out=po[:],
                        lhsT=vrow_sb[:, NW + bb * P : NW + (bb + 1) * P],
                        rhs=vrow_sb[:, 0:NW],
                        start=False,
                        stop=True,
                    )
                    sq = sqpool.tile([P, WIDTH], f32, tag="sq")
                    ssq = stat.tile([P, 1], f32, tag="ssq")
                    nc.scalar.activation(
                        out=sq[:], in_=po[:, :WIDTH], func=Act.Square, accum_out=ssq[:]
                    )
                    m2 = stat.tile([P, 1], f32, tag="m2")
                    nc.scalar.activation(
                        out=m2[:],
                        in_=po[:, WIDTH : WIDTH + 1],
                        func=Act.Square,
                        scale=1.0 / WIDTH,
                    )
                    rv = stat.tile([P, 1], f32, tag="rv")
                    nc.vector.tensor_scalar(
                        out=rv[:],
                        in0=ssq[:],
                        scalar1=1.0 / WIDTH,
                        scalar2=m2[:, :1],
                        op0=Alu.mult,
                        op1=Alu.subtract,
                    )
                    sd = stat.tile([P, 1], f32, tag="sd")
                    nc.scalar.activation(
                        out=sd[:], in_=rv[:], func=Act.Sqrt, bias=fcon_sb[:, bb : bb + 1]
                    )
                    rstd = stat.tile([P, 1], f32, tag="rstd")
                    nc.vector.reciprocal(rstd[:], sd[:])
                    nb = stat.tile([P, 1], f32, tag="nb")
                    nc.vector.tensor_scalar(
                        out=nb[:],
                        in0=po[:, WIDTH : WIDTH + 1],
                        scalar1=-1.0 / WIDTH,
                        scalar2=rstd[:, :1],
                        op0=Alu.mult,
                        op1=Alu.mult,
                    )
                    yo = ypool.tile([P, WIDTH], f32, tag="yo")
                    if not generic_affine:
                        nc.scalar.activation(
                            out=yo[:],
                            in_=po[:, :WIDTH],
                            func=Act.Relu,
                            scale=rstd[:, :1],
                            bias=nb[:, :1],
                        )
                    else:
                        t1 = ypool.tile([P, WIDTH], f32, tag="t1")
                        nc.scalar.activation(
                            out=t1[:],
                            in_=po[:, :WIDTH],
                            func=Act.Identity,
                            scale=rstd[:, :1],
                            bias=nb[:, :1],
                        )
                        t2 = ypool.tile([P, WIDTH], f32, tag="t2")
                        nc.vector.tensor_tensor(
                            out=t2[:], in0=t1[:], in1=gb_sb[:, :WIDTH], op=Alu.mult
                        )
                        t3 = ypool.tile([P, WIDTH], f32, tag="t3")
                        nc.vector.tensor_tensor(
                            out=t3[:], in0=t2[:], in1=gb_sb[:, WIDTH:], op=Alu.add
                        )
                        nc.scalar.activation(out=yo[:], in_=t3[:], func=Act.Relu)
                    rows = min(P, NODES_PER_CORE - bb * P)
                    nc.sync.dma_start(out_d[bb * P : bb * P + rows, :], yo[:rows, :])
    return nc


def _pack_inputs(meta, deg, idxe, idxo, scon, x, W, bias, gamma, beta, generic_affine):
    bfnp = ml_dtypes.bfloat16

    dinv = (1.0 / np.sqrt(deg)).astype(np.float64)
    xs = (np.asarray(x, np.float64) * dinv[:, None]).astype(bfnp)
    xe = np.ascontiguousarray(xs[0::2])
    xo = np.ascontiguousarray(xs[1::2])

    xself_all = np.zeros((N_CORES, N_BLOCKS, P, WIDTH), bfnp)
    for c in range(N_CORES):
        sl = xs[c * NODES_PER_CORE : (c + 1) * NODES_PER_CORE]
        flat = np.zeros((N_BLOCKS * P, WIDTH), bfnp)
        flat[: NODES_PER_CORE] = sl
        xself_all[c] = flat.reshape(N_BLOCKS, P, WIDTH)
    xself_all = np.ascontiguousarray(
        xself_all.transpose(0, 2, 1, 3).reshape(N_CORES, P, N_BLOCKS * WIDTH)
    )

    degp = np.ones((N_CORES, N_BLOCKS * P), np.float64)
    for c in range(N_CORES):
        degp[c, :NODES_PER_CORE] = deg[c * NODES_PER_CORE : (c + 1) * NODES_PER_CORE]
    epsdeg = (LN_EPS * degp).astype(np.float32).reshape(N_CORES, N_BLOCKS, P)
    epsdeg = np.ascontiguousarray(epsdeg.transpose(0, 2, 1))

    WT32 = np.asarray(W, np.float64).T
    rs = WT32.sum(axis=1, keepdims=True)
    WTe = np.concatenate([WT32, rs], axis=1).astype(bfnp)
    wt = np.concatenate([WTe[:P], WTe[P:]], axis=1)
    ident = np.eye(P, dtype=bfnp)
    wcon = np.ascontiguousarray(np.concatenate([wt, ident], axis=1))

    b64 = np.asarray(bias, np.float64)
    brow = np.concatenate([b64, [b64.sum()]])
    sdeg = np.sqrt(degp)
    vrow_all = np.concatenate(
        [np.tile(brow[None, :], (N_CORES, 1)), sdeg], axis=1
    ).astype(bfnp)

    in_maps = []
    for c in range(N_CORES):
        m = {
            "xe": xe,
            "xo": xo,
            "idxe": np.ascontiguousarray(idxe[c]),
            "idxo": np.ascontiguousarray(idxo[c]),
            "scon": np.ascontiguousarray(scon[c]),
            "xself": xself_all[c],
            "fcon": epsdeg[c],
            "wcon": wcon,
            "vrow": vrow_all[c : c + 1],
        }
        if generic_affine:
            gb = np.concatenate(
                [
                    np.tile(np.asarray(gamma, np.float32)[None, :], (P, 1)),
                    np.tile(np.asarray(beta, np.float32)[None, :], (P, 1)),
                ],
                axis=1,
            )
            m["gbcon"] = np.ascontiguousarray(gb)
        in_maps.append(m)
    return in_maps


_PROGRAM_CACHE = {}


def kernel(x, edge_index, W, b, gamma, beta, _run_kwargs=None):
    from concourse.bass_utils import run_bass_kernel_spmd

    x = np.asarray(x)
    W = np.asarray(W)
    bias = np.asarray(b)
    gamma = np.asarray(gamma)
    beta = np.asarray(beta)

    meta, deg, idxe, idxo, scon = _preprocess(edge_index)
    generic_affine = not (np.all(gamma == 1.0) and np.all(beta == 0.0))

    key = (meta, generic_affine)
    if key not in _PROGRAM_CACHE:
        nc = _build_program(meta, generic_affine)
        nc.finalize()
        _PROGRAM_CACHE[key] = nc
    nc = _PROGRAM_CACHE[key]

    in_maps = _pack_inputs(
        meta, deg, idxe, idxo, scon, x, W, bias, gamma, beta, generic_affine
    )

    kwargs = dict(_run_kwargs or {})
    kwargs.pop("_result", None)
    rr = run_bass_kernel_spmd(nc, in_maps, list(range(N_CORES)), **kwargs)
    out = np.concatenate([rr.results[c]["out"] for c in range(N_CORES)], axis=0)
    if _run_kwargs is not None:
        _run_kwargs["_result"] = rr
    return np.ascontiguousarray(out.astype(np.float32))
